# revision 1
# baseline (speedup 1.0000x reference)
"""Trainium2 Bass kernel for nn_Attention_58360015618558 (final).

Strategy (8 NeuronCores, SPMD). Measured 828 us (baseline 966 us),
rel err 7.8e-3.
  - Shard: core c -> (batch b = c//2, seq-half h = c%2); K/V computed for the
    local half and pair-AllGathered (2 MB each, hidden under compute).
  - Q/K/Kc projections are emitted DIRECTLY TRANSPOSED (weights stationary,
    x^T moving) so no PE transposes or PSUM->SBUF copies are needed.
    LayerNorm runs in the transposed layout: means come precomputed from the
    host (linear in x), E[z^2] via 1/N-scaled ones-matmuls (partition
    broadcast-reduce); RoPE's pair swap is a DVE stream_shuffle with rstd
    folded into transposed cos/sin tiles.  Projection order
    Vc, K, V, Q, Kc so the collectives and Q's long epilogue hide under
    later PE work.
  - Attention in transposed-score layout (S^T tiles): exp batched over
    [128,1024] 2-bank PSUM tiles (halves the ACT-overhead); softmax
    denominator via DVE pair-adds to 3 partials + ones-matmuls on PE; the
    caption denominator reuses the same PSUM bank.
  - Output projection interleaved per q-chunk into the attention loop (fills
    the PE while the scalar engine runs exp); output stored bf16, host upcast.
  - fp8 DoubleRow projections were measured at 5.2e-2 rel err (near-uniform
    attention passes q/k element noise straight through) — disabled.
"""

import math
import sys

import numpy as np

sys.path.insert(0, "/opt/trn_rl_repo")

import ml_dtypes  # noqa: E402

BF16 = ml_dtypes.bfloat16

# Full-size problem config
HID, H, KV, D, CAP = 2048, 16, 8, 128, 2048
B, S, LC = 4, 2048, 256
EPS = 1e-5
NCORES = 8

FULL_CFG = dict(S=S, SQ=S // 2, HID=HID, CAP=CAP, LC=LC, H=H, KV=KV)


DEBUG_DUMPS = False
# fp8e4 DoubleRow projections measured 5.2e-2 rel err (near-uniform attention
# probabilities pass q/k element noise straight to the output) — keep off.
FP8_PROJ = False


def _build(cfg, gate_t, ln_trivial=(True, True, True)):
    """Build the per-core Bass program. Returns compiled Bacc."""
    import concourse.bass as bass  # noqa: F401
    import concourse.mybir as mybir
    import concourse.tile as tile
    from concourse import bacc
    from contextlib import ExitStack

    FP = mybir.dt.float32
    BF = mybir.dt.bfloat16
    F8 = mybir.dt.float8e4
    WDT = F8 if FP8_PROJ else BF
    DR = mybir.MatmulPerfMode.DoubleRow
    AF = mybir.ActivationFunctionType
    ALU = mybir.AluOpType

    S_, SQ, HID_, CAP_, LC_ = cfg["S"], cfg["SQ"], cfg["HID"], cfg["CAP"], cfg["LC"]
    H_, KV_ = cfg["H"], cfg["KV"]
    HD, KD = H_ * D, KV_ * D
    CT, CTC = HID_ // 128, CAP_ // 128
    NQ, NK, NLC = SQ // 128, S_ // 128, LC_ // 128
    SCALE = 1.0 / math.sqrt(D)
    qtriv, ktriv, kctriv = ln_trivial
    SWAPMASK = [i ^ 1 for i in range(32)]

    nc = bacc.Bacc("TRN2", target_bir_lowering=False, debug=False,
                   num_devices=NCORES)

    def din(name, shape, dt=BF):
        return nc.dram_tensor(name, shape, dt, kind="ExternalInput").ap()

    xT = din("xT", [HID_, SQ])          # x[b].T columns of this half
    capT = din("capT", [CAP_, LC_])
    # packed transposed-proj weights: [dt, p, ct, q]
    wq = din("wq", [HD // 128, 128, CT, 128], WDT)
    wk = din("wk", [KD // 128, 128, CT, 128], WDT)
    wkc = din("wkc", [KD // 128, 128, CTC, 128], WDT)
    if FP8_PROJ:
        xT8 = din("xT8", [HID_, SQ], F8)
        capT8 = din("capT8", [CAP_, LC_], F8)
    # natural-proj weights; wv packed as quarters [i, p, ct, 256]
    wv = din("wv", [4, 128, CT, KD // 4])
    wvc = din("wvc", [CAP_, KD])
    wo = din("wo", [HD, HID_])
    cosT = din("cosT", [128, SQ])       # cosT[d,s] = cos[s, d//2]
    sinT = din("sinT", [128, SQ])       # signed: -sin even d, +sin odd d
    # LN means, precomputed host-side (linear in x) and pre-broadcast
    m_q = din("m_q", [128, SQ])
    m_k = din("m_k", [128, SQ])
    m_kc = din("m_kc", [128, LC_])
    lnw = {}
    for nm, dflat in (("q", HD), ("k", KD), ("kc", KD)):
        lnw[nm] = (din(f"ln_{nm}_w", [dflat], FP), din(f"ln_{nm}_b", [dflat], FP))
    out = nc.dram_tensor("out", [SQ, HID_], BF, kind="ExternalOutput").ap()

    with ExitStack() as top:
        tc = top.enter_context(tile.TileContext(nc))

        constp = top.enter_context(tc.tile_pool(name="const", bufs=1))
        resp = top.enter_context(tc.tile_pool(name="res", bufs=1))
        dramp = top.enter_context(tc.tile_pool(name="dram", bufs=1, space="DRAM"))

        ones_q = constp.tile([128, 128], BF, tag="ones_q", name="ones_q")
        nc.vector.memset(ones_q[:], 1.0 / HD)
        ones_k = constp.tile([128, 128], BF, tag="ones_k", name="ones_k")
        nc.vector.memset(ones_k[:], 1.0 / KD)
        ones_1 = constp.tile([128, 128], BF, tag="ones_1", name="ones_1")
        nc.vector.memset(ones_1[:], 1.0)
        zero_c = constp.tile([128, 1], FP, tag="zero_c", name="zero_c")
        nc.vector.memset(zero_c[:], 0.0)
        nc.const_aps.aps[(FP, 0.0)] = zero_c[:]
        eps_c = constp.tile([128, 1], FP, tag="eps_c", name="eps_c")
        nc.vector.memset(eps_c[:], EPS)
        nc.const_aps.aps[(FP, EPS)] = eps_c[:]

        cos_res = constp.tile([128, SQ], BF, tag="cos_res", name="cos_res")
        sin_res = constp.tile([128, SQ], BF, tag="sin_res", name="sin_res")
        nc.sync.dma_start(cos_res[:], cosT)
        nc.sync.dma_start(sin_res[:], sinT)
        mq_res = constp.tile([128, SQ], BF, tag="mq_res", name="mq_res")
        mk_res = constp.tile([128, SQ], BF, tag="mk_res", name="mk_res")
        mkc_res = constp.tile([128, LC_], BF, tag="mkc_res", name="mkc_res")
        nc.sync.dma_start(mq_res[:], m_q)
        nc.sync.dma_start(mk_res[:], m_k)
        nc.sync.dma_start(mkc_res[:], m_kc)

        # LN affine params in transposed layout: [128, n_dt] (col dt = head
        # tile), applied per-partition. Only loaded when nontrivial.
        affs = {}
        for nm, dflat, triv in (("q", HD, qtriv), ("k", KD, ktriv),
                                ("kc", KD, kctriv)):
            if not triv:
                wsb = constp.tile([128, dflat // 128], FP, tag=f"aw_{nm}",
                                  name=f"aw_{nm}")
                bsb = constp.tile([128, dflat // 128], FP, tag=f"ab_{nm}",
                                  name=f"ab_{nm}")
                nc.sync.dma_start(wsb[:], lnw[nm][0].rearrange("(o p) -> p o", p=128))
                nc.sync.dma_start(bsb[:], lnw[nm][1].rearrange("(o p) -> p o", p=128))
                affs[nm] = (wsb, bsb)

        # Resident tensors
        V_res = resp.tile([128, NK, KD], BF, tag="V_res", name="V_res")
        Vc_res = resp.tile([128, NLC, KD], BF, tag="Vc_res", name="Vc_res")
        KcT_res = resp.tile([128, KV_, LC_], BF, tag="KcT_res", name="KcT_res")
        QT_res = resp.tile([128, H_, SQ], BF, tag="QT_res", name="QT_res")

        # DRAM intermediates
        KVD = KV_ * 128
        KT_loc = dramp.tile([KVD, SQ], BF, tag="KT_loc", name="KT_loc")
        KT_g = dramp.tile([2 * KVD, SQ], BF, tag="KT_g", name="KT_g")
        V_loc = dramp.tile([SQ, KD], BF, tag="V_loc", name="V_loc")
        V_g = dramp.tile([2 * SQ, KD], BF, tag="V_g", name="V_g")
        aT = dramp.tile([H_, 128, SQ], BF, tag="aT", name="aT")
        aT_r = aT.rearrange("h p s -> p h s")
        dumps = {}
        if DEBUG_DUMPS:
            for nm, shp in (("d_qt", [H_, 128, SQ]), ("d_ktg", [2 * KVD, SQ]),
                            ("d_vg", [2 * SQ, KD]), ("d_at", [H_, 128, SQ]),
                            ("d_kct", [KV_, 128, LC_])):
                dumps[nm] = nc.dram_tensor(nm, shp, BF, kind="ExternalOutput").ap()
            for nm, shp, dt_ in (("d_rden", [128, 512], FP),
                                 ("d_rdenc", [128, 512], FP),
                                 ("d_t2", [128, 512], FP),
                                 ("d_tmp", [128, 512], FP),
                                 ("d_l2", [128, 4, 512], BF),
                                 ("d_pt0", [128, 1024], BF),
                                 ("d_ptc", [128, 1024], BF),
                                 ("d_denc", [128, 512], FP),
                                 ("d_poc", [128, 512], FP)):
                dumps[nm] = nc.dram_tensor(nm, shp, dt_,
                                           kind="ExternalOutput").ap()

        # ------------- transposed projection with LN (+RoPE) -------------
        def tproj(P, src_res, n_ct, w_ap, n_dt, s_len, ones_t, m_res,
                  rope, dest_fn, aff, dma_eng=None):
            """dest_fn(dt, s0, CW) -> ("sbuf", AP) | ("dram", AP).
            P: dict of shared pools."""
            CW = min(512, s_len)
            NCH = s_len // CW
            wtp, zp, zsqp, pp, statp, scp, app, stgp = (
                P["wt"], P["zp"], P["zsq"], P["pjps"], P["statp"], P["pjsc"],
                P["app"], P["stg"])

            for ch in range(NCH):
                s0 = ch * CW
                z = zp.tile([128, n_dt, CW], BF, tag="z", name="z")
                m_bf = m_res[:, s0:s0 + CW]
                msq = scp.tile([128, CW], FP, tag="msq", name="msq")
                nc.scalar.activation(msq[:], m_bf, AF.Square)
                sqP = statp.tile([128, CW], FP, tag="sqP", name="sqP")
                for dt in range(n_dt):
                    wt = wtp.tile([128, n_ct, 128], WDT, tag="wt", name="wt")
                    nc.sync.dma_start(wt[:], w_ap[dt])
                    ps = pp.tile([128, CW], FP, tag="ps", name="ps")
                    if FP8_PROJ:
                        for c2 in range(n_ct // 2):
                            nc.tensor.matmul(
                                ps[:], lhsT=wt[:, 2 * c2:2 * c2 + 2, :],
                                rhs=src_res[:, 2 * c2:2 * c2 + 2, s0:s0 + CW],
                                start=(c2 == 0), stop=(c2 == n_ct // 2 - 1),
                                perf_mode=DR)
                    else:
                        for ct in range(n_ct):
                            nc.tensor.matmul(ps[:], lhsT=wt[:, ct, :],
                                             rhs=src_res[:, ct, s0:s0 + CW],
                                             start=(ct == 0),
                                             stop=(ct == n_ct - 1))
                    nc.scalar.copy(z[:, dt, :], ps[:])
                    zq = zsqp.tile([128, CW], BF, tag="zq", name="zq")
                    nc.vector.tensor_tensor(zq[:], z[:, dt, :], z[:, dt, :],
                                            ALU.mult)
                    nc.tensor.matmul(sqP[:], lhsT=ones_t[:], rhs=zq[:],
                                     start=(dt == 0), stop=(dt == n_dt - 1))
                # stats: var = E[z^2] - m^2 (ones pre-scaled by 1/div)
                var = scp.tile([128, CW], FP, tag="var", name="var")
                nc.vector.tensor_tensor(var[:], sqP[:], msq[:], ALU.subtract)
                sd = scp.tile([128, CW], FP, tag="sd", name="sd")
                nc.scalar.activation(sd[:], var[:], AF.Sqrt, bias=EPS)
                rs = scp.tile([128, CW], FP, tag="rs", name="rs")
                nc.vector.reciprocal_approx_fast(rs[:], sd[:])
                if aff is None:
                    if rope:
                        RC = scp.tile([128, CW], BF, tag="RC", name="RC")
                        nc.vector.tensor_tensor(RC[:], rs[:],
                                                cos_res[:, s0:s0 + CW], ALU.mult)
                        RS = scp.tile([128, CW], BF, tag="RS", name="RS")
                        nc.vector.tensor_tensor(RS[:], rs[:],
                                                sin_res[:, s0:s0 + CW], ALU.mult)
                    else:
                        rs_bf = scp.tile([128, CW], BF, tag="rs_bf", name="rs_bf")
                        nc.vector.tensor_copy(rs_bf[:], rs[:])
                # zc pass first: frees z (and its SBUF region) early, before
                # the longer rope chains run on DVE
                zcs = []
                for dt in range(n_dt):
                    zc = app.tile([128, CW], BF, tag="zc", name="zc", bufs=16)
                    nc.vector.tensor_tensor(zc[:], z[:, dt, :], m_bf,
                                            ALU.subtract)
                    zcs.append(zc)
                for dt in range(n_dt):
                    zc = zcs[dt]
                    mode, dest = dest_fn(dt, s0, CW)
                    if mode == "dram":
                        o = stgp.tile([128, CW], BF, tag="o", name="o")
                        oap = o[:]
                    else:
                        oap = dest
                    if aff is not None:
                        # full chain: zn=zc*rs; za=zn*w+b; then rope
                        zn = app.tile([128, CW], BF, tag="zn", name="zn")
                        nc.vector.tensor_tensor(zn[:], zc[:], rs[:], ALU.mult)
                        za = app.tile([128, CW], BF, tag="za", name="za")
                        wsb, bsb = aff
                        nc.vector.tensor_scalar(za[:], zn[:],
                                                wsb[:, dt:dt + 1],
                                                bsb[:, dt:dt + 1],
                                                ALU.mult, ALU.add)
                        if rope:
                            sh = app.tile([128, CW], BF, tag="sh", name="sh")
                            nc.vector.stream_shuffle(sh[:], za[:], SWAPMASK)
                            t1 = app.tile([128, CW], BF, tag="t1", name="t1")
                            nc.vector.tensor_tensor(t1[:], za[:],
                                                    cos_res[:, s0:s0 + CW],
                                                    ALU.mult)
                            t2 = app.tile([128, CW], BF, tag="t2", name="t2")
                            nc.vector.tensor_tensor(t2[:], sh[:],
                                                    sin_res[:, s0:s0 + CW],
                                                    ALU.mult)
                            nc.vector.tensor_tensor(oap, t1[:], t2[:], ALU.add)
                        else:
                            nc.vector.tensor_copy(oap, za[:])
                    elif rope:
                        sh = app.tile([128, CW], BF, tag="sh", name="sh")
                        nc.vector.stream_shuffle(sh[:], zc[:], SWAPMASK)
                        t1 = app.tile([128, CW], BF, tag="t1", name="t1")
                        nc.vector.tensor_tensor(t1[:], zc[:], RC[:], ALU.mult)
                        t2 = app.tile([128, CW], BF, tag="t2", name="t2")
                        nc.vector.tensor_tensor(t2[:], sh[:], RS[:], ALU.mult)
                        nc.vector.tensor_tensor(oap, t1[:], t2[:], ALU.add)
                    else:
                        nc.vector.tensor_tensor(oap, zc[:], rs_bf[:], ALU.mult)
                    if mode == "dram":
                        (dma_eng or nc.sync).dma_start(dest, o[:])

        # ------------- natural projection (V / Vc), no LN -------------
        def nproj(P, src_res, n_ct, w_res, n_st, dest_fn):
            pp, stg = P["vps"], P["vstg"]
            for st in range(n_st):
                for i in range(KD // 512):
                    ps = pp.tile([128, 512], FP, tag="ps", name="ps")
                    for ct in range(n_ct):
                        nc.tensor.matmul(
                            ps[:], lhsT=src_res[:, ct, st * 128:(st + 1) * 128],
                            rhs=w_res[:, ct, i * 512:(i + 1) * 512],
                            start=(ct == 0), stop=(ct == n_ct - 1))
                    mode, dest = dest_fn(st, i)
                    if mode == "dram":
                        v = stg.tile([128, 512], BF, tag="v", name="v")
                        nc.scalar.copy(v[:], ps[:])
                        nc.sync.dma_start(dest, v[:])
                    else:
                        nc.scalar.copy(dest, ps[:])

        # ktw pool lives at top scope so phase-B K loads never wait on
        # phase-A SBUF frees; first two kv tiles are preloaded during A.
        kp = top.enter_context(tc.tile_pool(name="kw", bufs=2))

        def load_ktw(kv, eng=None):
            # eng=gpsimd for phase-A preloads: they depend on the K gather
            # and must not head-of-line-block the Sync queue
            eng = eng or nc.sync
            ktw = kp.tile([128, S_], BF, tag="ktw", name="ktw")
            eng.dma_start(ktw[:, 0:SQ], KT_g[kv * 128:(kv + 1) * 128, :])
            eng.dma_start(ktw[:, SQ:2 * SQ],
                          KT_g[KVD + kv * 128:KVD + (kv + 1) * 128, :])
            return ktw

        # ---------------- Phase A ----------------
        RG = [[2 * i, 2 * i + 1] for i in range(NCORES // 2)]
        ktw_pre = {}
        with ExitStack() as pa:
            # shared projection pools (one open/close for all of phase A)
            P = {}
            P["wt"] = pa.enter_context(tc.tile_pool(name="wt", bufs=2))
            P["zp"] = pa.enter_context(tc.tile_pool(name="zp", bufs=1))
            # (zp bufs=1 is safe: the zc-first pass frees z early each chunk)
            P["zsq"] = pa.enter_context(tc.tile_pool(name="zsq", bufs=2))
            P["pjps"] = pa.enter_context(tc.tile_pool(name="pjps", bufs=4,
                                                      space="PSUM"))
            P["statp"] = pa.enter_context(tc.tile_pool(name="statp", bufs=1,
                                                       space="PSUM"))
            P["pjsc"] = pa.enter_context(tc.tile_pool(name="pjsc", bufs=1))
            P["app"] = pa.enter_context(tc.tile_pool(name="app", bufs=2))
            P["stg"] = pa.enter_context(tc.tile_pool(name="stg", bufs=2))
            P["vps"] = pa.enter_context(tc.tile_pool(name="vps", bufs=3,
                                                     space="PSUM"))
            P["vstg"] = pa.enter_context(tc.tile_pool(name="vstg", bufs=3))

            # caption features stay resident: the (small) Kc projection runs
            # LAST so its short epilogue — not Q's long one — gates attention
            ctp = pa.enter_context(tc.tile_pool(name="ct", bufs=1))
            cap_res = ctp.tile([128, CTC, LC_], BF, tag="cap", name="cap")
            cap_view = capT.rearrange("(co p) s -> p co s", p=128)
            if FP8_PROJ:
                cap8_res = ctp.tile([128, CTC, LC_], F8, tag="cap8",
                                    name="cap8")
                cap8_view = capT8.rearrange("(co p) s -> p co s", p=128)
                for ct in range(CTC):
                    nc.sync.dma_start(cap8_res[:, ct, :], cap8_view[:, ct, :])
                kc_src = cap8_res
            else:
                kc_src = cap_res

            with ExitStack() as s1:   # caption V weights
                wvcp = s1.enter_context(tc.tile_pool(name="wvcp", bufs=1))
                wvc_res = wvcp.tile([128, CTC, KD], BF, tag="wvc", name="wvc")
                wvc_view = wvc.rearrange("(co p) d -> p co d", p=128)
                for ct in range(CTC):
                    nc.sync.dma_start(cap_res[:, ct, :], cap_view[:, ct, :])
                    nc.sync.dma_start(wvc_res[:, ct, :], wvc_view[:, ct, :])
                nproj(P, cap_res, CTC, wvc_res, NLC,
                      lambda st, i: ("sbuf",
                                     Vc_res[:, st, i * 512:(i + 1) * 512]))

            with ExitStack() as s2:   # x^T (fp8 when enabled), through Q proj
                xtp = s2.enter_context(tc.tile_pool(name="xtq", bufs=1))
                xT_view = xT.rearrange("(co p) s -> p co s", p=128)
                if FP8_PROJ:
                    x_src = xtp.tile([128, CT, SQ], F8, tag="xt8", name="xt8")
                    xT8_view = xT8.rearrange("(co p) s -> p co s", p=128)
                    for ct in range(CT):
                        nc.sync.dma_start(x_src[:, ct, :], xT8_view[:, ct, :])
                else:
                    x_src = xtp.tile([128, CT, SQ], BF, tag="xt", name="xt")
                    for ct in range(CT):
                        nc.sync.dma_start(x_src[:, ct, :], xT_view[:, ct, :])
                tproj(P, x_src, CT, wk, KV_, SQ, ones_k, mk_res, rope=True,
                      dest_fn=lambda dt, s0, CW: (
                          "dram", KT_loc[dt * 128:(dt + 1) * 128, s0:s0 + CW]),
                      aff=affs.get("k"))
                nc.gpsimd.collective_compute(
                    "AllGather", ALU.bypass, replica_groups=RG,
                    ins=[KT_loc.opt()], outs=[KT_g.opt()])
                for kvp in (0, 1):
                    ktw_pre[(0, kvp)] = load_ktw(kvp, eng=nc.gpsimd)
                with ExitStack() as s3:   # V projection
                    wvp = s3.enter_context(tc.tile_pool(name="wvp", bufs=1))
                    if FP8_PROJ:
                        # wv resident, bf16 x tiles streamed per s-tile
                        wv_res = wvp.tile([128, CT, KD], BF, tag="wv",
                                          name="wv")
                        wv_rv = wv_res.rearrange("p ct (i q) -> p ct i q", i=4)
                        for i4 in range(4):
                            nc.sync.dma_start(wv_rv[:, :, i4, :], wv[i4])
                        xtvp = s3.enter_context(tc.tile_pool(name="xtv",
                                                             bufs=2))
                        for st in range(NQ):
                            xtv = xtvp.tile([128, CT, 128], BF, tag="xtv",
                                            name="xtv")
                            nc.sync.dma_start(
                                xtv[:], xT_view[:, :, st * 128:(st + 1) * 128])
                            for i in range(KD // 512):
                                ps = P["vps"].tile([128, 512], FP, tag="ps",
                                                   name="ps")
                                for ct in range(CT):
                                    nc.tensor.matmul(
                                        ps[:], lhsT=xtv[:, ct, :],
                                        rhs=wv_res[:, ct,
                                                   i * 512:(i + 1) * 512],
                                        start=(ct == 0), stop=(ct == CT - 1))
                                v = P["vstg"].tile([128, 512], BF, tag="v",
                                                   name="v")
                                nc.scalar.copy(v[:], ps[:])
                                nc.gpsimd.dma_start(
                                    V_loc[st * 128:(st + 1) * 128,
                                          i * 512:(i + 1) * 512], v[:])
                    else:
                        # x^T resident (bf16), wv streamed in quarters
                        for i in range(4):
                            wvh = wvp.tile([128, CT, 256], BF, tag="wvh",
                                           name="wvh", bufs=2)
                            nc.sync.dma_start(wvh[:], wv[i])
                            for st in range(NQ):
                                ps = P["vps"].tile([128, 256], FP, tag="ps",
                                                   name="ps")
                                for ct in range(CT):
                                    nc.tensor.matmul(
                                        ps[:],
                                        lhsT=x_src[:, ct,
                                                   st * 128:(st + 1) * 128],
                                        rhs=wvh[:, ct, :],
                                        start=(ct == 0), stop=(ct == CT - 1))
                                v = P["vstg"].tile([128, 256], BF, tag="v",
                                                   name="v")
                                nc.scalar.copy(v[:], ps[:])
                                nc.gpsimd.dma_start(
                                    V_loc[st * 128:(st + 1) * 128,
                                          i * 256:(i + 1) * 256], v[:])
                    nc.gpsimd.collective_compute(
                        "AllGather", ALU.bypass, replica_groups=RG,
                        ins=[V_loc.opt()], outs=[V_g.opt()])
                    # fills depend on the collective: off the Sync queue
                    for b2 in range(2):
                        for stl in range(NQ):
                            nc.gpsimd.dma_start(
                                V_res[:, b2 * NQ + stl, :],
                                V_g[b2 * SQ + stl * 128:
                                    b2 * SQ + (stl + 1) * 128, :])
                tproj(P, x_src, CT, wq, H_, SQ, ones_q, mq_res, rope=True,
                      dest_fn=lambda dt, s0, CW: ("sbuf",
                                                  QT_res[:, dt, s0:s0 + CW]),
                      aff=affs.get("q"))
            # Kc last: its short epilogue gates attention start, while Q's
            # long rope epilogue drains during Kc's PE work
            tproj(P, kc_src, CTC, wkc, KV_, LC_, ones_k, mkc_res, rope=False,
                  dest_fn=lambda dt, s0, CW: ("sbuf",
                                              KcT_res[:, dt, s0:s0 + CW]),
                  aff=affs.get("kc"))
            # dummy exp: pulls the Sqrt->Exp ACT table-set switch (~2.7us)
            # into scalar-idle time instead of attention's first exp
            warm = constp.tile([128, 1], FP, tag="warm", name="warm")
            nc.scalar.activation(warm[:], zero_c[:], AF.Exp)

        if DEBUG_DUMPS:
            for hh in range(H_):
                nc.sync.dma_start(dumps["d_qt"][hh], QT_res[:, hh, :])
            for kk in range(KV_):
                nc.sync.dma_start(dumps["d_kct"][kk], KcT_res[:, kk, :])
            for kt_ in range(NK):
                nc.sync.dma_start(dumps["d_vg"][kt_ * 128:(kt_ + 1) * 128, :],
                                  V_res[:, kt_, :])

        # ---------------- Phase B + interleaved C ----------------
        QCH = 512
        NQC = SQ // QCH
        wop = top.enter_context(tc.tile_pool(name="wop", bufs=1))
        wo_res = wop.tile([128, H_, HID_], BF, tag="wo", name="wo")
        wo_view = wo.rearrange("(ho p) e -> p ho e", p=128)
        for hh in range(H_):
            # gpsimd DMA queue: keeps these off the Sync queue's critical
            # path into attention (only needed by phase C)
            nc.gpsimd.dma_start(wo_res[:, hh, :], wo_view[:, hh, :])
        with ExitStack() as pb:
            ptp = pb.enter_context(tc.tile_pool(name="pt", bufs=3))
            l1p = pb.enter_context(tc.tile_pool(name="l1", bufs=4))
            l2p = pb.enter_context(tc.tile_pool(name="l2", bufs=8))
            epi = pb.enter_context(tc.tile_pool(name="epi", bufs=2))
            aop = pb.enter_context(tc.tile_pool(name="ao", bufs=2))
            ps_s = pb.enter_context(tc.tile_pool(name="ps_s", bufs=2, space="PSUM"))
            ps_o = pb.enter_context(tc.tile_pool(name="ps_o", bufs=1, space="PSUM"))
            ps_oc = pb.enter_context(tc.tile_pool(name="ps_oc", bufs=1,
                                                  space="PSUM"))
            ps_d = pb.enter_context(tc.tile_pool(name="ps_d", bufs=1, space="PSUM"))
            # phase C pools
            ap_ = pb.enter_context(tc.tile_pool(name="ast", bufs=2))
            op_ = pb.enter_context(tc.tile_pool(name="osb", bufs=2))
            cps = pb.enter_context(tc.tile_pool(name="cps", bufs=1, space="PSUM"))

            last_ao = [None]
            for ch in range(NQC):
                q0 = ch * QCH
                for kv in range(KV_):
                    ktw = ktw_pre.pop((ch, kv), None)
                    if ktw is None:
                        ktw = load_ktw(kv)
                    if DEBUG_DUMPS and ch == 0:
                        nc.sync.dma_start(
                            dumps["d_ktg"][kv * 128:(kv + 1) * 128, :],
                            ktw[:, 0:SQ])
                        nc.sync.dma_start(
                            dumps["d_ktg"][KVD + kv * 128:KVD + (kv + 1) * 128, :],
                            ktw[:, SQ:2 * SQ])
                    for rep in range(H_ // KV_):
                        h = kv * (H_ // KV_) + rep
                        qs = QT_res[:, h, q0:q0 + QCH]
                        po = ps_o.tile([128, QCH], FP, tag="po", name="po")
                        poc = ps_oc.tile([128, QCH], FP, tag="poc", name="poc")
                        pden = ps_d.tile([128, QCH], FP, tag="pden", name="pden")
                        l2s = []
                        for g in range(NK // 2):
                            ps2 = ps_s.tile([128, 2 * QCH], FP, tag="s", name="s")
                            nc.tensor.matmul(ps2[:, 0:QCH],
                                             lhsT=ktw[:, (2 * g) * 128:
                                                      (2 * g + 1) * 128],
                                             rhs=qs, start=True, stop=True)
                            nc.tensor.matmul(ps2[:, QCH:2 * QCH],
                                             lhsT=ktw[:, (2 * g + 1) * 128:
                                                      (2 * g + 2) * 128],
                                             rhs=qs, start=True, stop=True)
                            pt2 = ptp.tile([128, 2 * QCH], BF, tag="pt", name="pt")
                            nc.scalar.activation(pt2[:], ps2[:], AF.Exp,
                                                 scale=SCALE)
                            nc.tensor.matmul(
                                po[:], lhsT=V_res[:, 2 * g,
                                                  kv * 128:(kv + 1) * 128],
                                rhs=pt2[:, 0:QCH], start=(g == 0), stop=False)
                            nc.tensor.matmul(
                                po[:], lhsT=V_res[:, 2 * g + 1,
                                                  kv * 128:(kv + 1) * 128],
                                rhs=pt2[:, QCH:2 * QCH], start=False,
                                stop=(g == NK // 2 - 1))
                            if DEBUG_DUMPS and h == 0 and ch == 0 and g == 0:
                                nc.sync.dma_start(dumps["d_pt0"], pt2[:])
                            l1 = l1p.tile([128, QCH], BF, tag="l1", name="l1")
                            nc.vector.tensor_tensor(l1[:], pt2[:, 0:QCH],
                                                    pt2[:, QCH:2 * QCH], ALU.add)
                            l2s.append(l1)
                            # combine pairs except the last one, so the pden
                            # matmuls needn't wait for the final DVE add
                            if g % 2 == 1 and g != NK // 2 - 1:
                                l2 = l2p.tile([128, QCH], BF, tag="l2", name="l2")
                                nc.vector.tensor_tensor(l2[:], l2s[-2][:],
                                                        l2s[-1][:], ALU.add)
                                l2s = l2s[:-2] + [None]
                                l2s[-1] = l2
                            # two extra mid-iteration combines (inputs already
                            # available) cut the pden matmuls from 5 to 3
                            if g in (3, 6) and len(l2s) >= 2:
                                l3 = l2p.tile([128, QCH], BF, tag="l2", name="l2")
                                nc.vector.tensor_tensor(l3[:], l2s[-2][:],
                                                        l2s[-1][:], ALU.add)
                                l2s = l2s[:-2] + [l3]
                        # pden = sum of the 4 level-2 partials: ones-matmuls
                        # broadcast-reduce over partitions (keeps the DVE
                        # epilogue chain short)
                        l2fin = [t for t in l2s if t is not None]
                        # caption scores first: they are independent of the
                        # DVE partial-sum tree, so the scalar engine gets its
                        # next exp sooner while pden waits on the tree
                        psc = ps_s.tile([128, 2 * QCH], FP, tag="s", name="s")
                        nc.tensor.matmul(psc[:, 0:QCH],
                                         lhsT=KcT_res[:, kv, 0:128],
                                         rhs=qs, start=True, stop=True)
                        nc.tensor.matmul(psc[:, QCH:2 * QCH],
                                         lhsT=KcT_res[:, kv, 128:256],
                                         rhs=qs, start=True, stop=True)
                        ptc = ptp.tile([128, 2 * QCH], BF, tag="pt", name="pt")
                        nc.scalar.activation(ptc[:], psc[:], AF.Exp, scale=SCALE)
                        for j, l2 in enumerate(l2fin):
                            nc.tensor.matmul(pden[:], lhsT=ones_1[:], rhs=l2[:],
                                             start=(j == 0),
                                             stop=(j == len(l2fin) - 1))
                        nc.tensor.matmul(poc[:],
                                         lhsT=Vc_res[:, 0, kv * 128:(kv + 1) * 128],
                                         rhs=ptc[:, 0:QCH], start=True, stop=False)
                        nc.tensor.matmul(poc[:],
                                         lhsT=Vc_res[:, 1, kv * 128:(kv + 1) * 128],
                                         rhs=ptc[:, QCH:2 * QCH], start=False,
                                         stop=True)
                        # caption pair-sum (elementwise); partition reduction
                        # happens below via a ones-matmul reusing pden's bank
                        denc = epi.tile([128, QCH], BF, tag="denc", name="denc")
                        nc.vector.tensor_tensor(denc[:], ptc[:, 0:QCH],
                                                ptc[:, QCH:2 * QCH], ALU.add)
                        if DEBUG_DUMPS and h == 0 and ch == 0:
                            nc.sync.dma_start(dumps["d_ptc"], ptc[:])
                            pocc = epi.tile([128, QCH], FP, tag="pocc",
                                            name="pocc")
                            nc.vector.tensor_copy(pocc[:], poc[:])
                            nc.sync.dma_start(dumps["d_poc"], pocc[:])
                        # epilogue
                        rden = epi.tile([128, QCH], FP, tag="rden", name="rden")
                        nc.vector.reciprocal_approx_fast(rden[:], pden[:])
                        nc.tensor.matmul(pden[:], lhsT=ones_1[:], rhs=denc[:],
                                         start=True, stop=True)
                        rdenc = epi.tile([128, QCH], FP, tag="rdenc", name="rdenc")
                        nc.vector.reciprocal_approx_fast(rdenc[:], pden[:])
                        if DEBUG_DUMPS and h == 0 and ch == 0:
                            nc.sync.dma_start(dumps["d_denc"], rdenc[:])
                        t2 = epi.tile([128, QCH], FP, tag="t2", name="t2")
                        nc.vector.tensor_tensor(t2[:], po[:], rden[:], ALU.mult)
                        tmp = epi.tile([128, QCH], FP, tag="tmp", name="tmp")
                        nc.vector.scalar_tensor_tensor(
                            tmp[:], poc[:], float(gate_t[h]), rdenc[:],
                            ALU.mult, ALU.mult)
                        if DEBUG_DUMPS and h == 0 and ch == 0:
                            nc.sync.dma_start(dumps["d_rden"], rden[:])
                            nc.sync.dma_start(dumps["d_rdenc"], rdenc[:])
                            nc.sync.dma_start(dumps["d_t2"], t2[:])
                            nc.sync.dma_start(dumps["d_tmp"], tmp[:])
                        ao = aop.tile([128, QCH], BF, tag="ao", name="ao")
                        nc.vector.tensor_tensor(ao[:], t2[:], tmp[:], ALU.add)
                        nc.sync.dma_start(aT[h, :, q0:q0 + QCH], ao[:])
                        if h == H_ - 1:
                            last_ao[0] = ao
                        if DEBUG_DUMPS:
                            nc.sync.dma_start(dumps["d_at"][h, :, q0:q0 + QCH],
                                              ao[:])
                # ---- phase C for this q-chunk ----
                # In the last chunk (nothing left to overlap), double-buffer
                # the accumulator by borrowing score-pool banks.
                tail = (ch == NQC - 1)
                for st in range(q0 // 128, (q0 + QCH) // 128):
                    a_st = ap_.tile([128, H_, 128], BF, tag="ast", name="ast")
                    if tail:
                        # last head's output read straight from SBUF: skips
                        # waiting on its DRAM round trip
                        nc.sync.dma_start(
                            a_st[:, 0:H_ - 1, :],
                            aT_r[:, 0:H_ - 1, st * 128:(st + 1) * 128])
                        o0 = st * 128 - q0
                        nc.vector.tensor_copy(a_st[:, H_ - 1, :],
                                              last_ao[0][:, o0:o0 + 128])
                    else:
                        nc.sync.dma_start(
                            a_st[:], aT_r[:, :, st * 128:(st + 1) * 128])
                    for ec in range(HID_ // 512):
                        if tail and ec % 2 == 1:
                            psfull = ps_s.tile([128, 2 * QCH], FP, tag="s",
                                               name="s")
                            ps = psfull[:, 0:512]
                        else:
                            ps = cps.tile([128, 512], FP, tag="cps",
                                          name="cps")[:]
                        for h in range(H_):
                            nc.tensor.matmul(ps, lhsT=a_st[:, h, :],
                                             rhs=wo_res[:, h,
                                                        ec * 512:(ec + 1) * 512],
                                             start=(h == 0), stop=(h == H_ - 1))
                        osb = op_.tile([128, 512], BF, tag="osb", name="osb")
                        nc.vector.tensor_copy(osb[:], ps)
                        nc.sync.dma_start(
                            out[st * 128:(st + 1) * 128, ec * 512:(ec + 1) * 512],
                            osb[:])

    nc.compile()
    return nc


_CACHE = {}


def _get_program(cfg, gate_t, ln_trivial):
    key = (tuple(sorted(cfg.items())), tuple(np.round(gate_t, 8)), ln_trivial,
           FP8_PROJ)
    if key not in _CACHE:
        _CACHE[key] = _build(cfg, gate_t, ln_trivial)
    return _CACHE[key]


def make_in_maps(cfg, inputs):
    """Host-side sharding: returns (in_maps, gate_t, ln_trivial)."""
    S_, SQ = cfg["S"], cfg["SQ"]
    x = np.asarray(inputs["x"], np.float32)
    cap = np.asarray(inputs["caption_feat"], np.float32)
    cos = np.ascontiguousarray(np.asarray(inputs["freqs_cos"], np.float32))
    sin = np.ascontiguousarray(np.asarray(inputs["freqs_sin"], np.float32))
    gate_t = np.tanh(np.asarray(inputs["gate"], np.float32))

    F8 = ml_dtypes.float8_e4m3

    def bf(a):
        return np.ascontiguousarray(a).astype(BF16)

    def fp8(a, scale=1.0):
        return np.ascontiguousarray(
            np.clip(a * scale, -440.0, 440.0)).astype(F8)

    def pack_t(w, n_dt, n_ct):
        # [n_ct*128, n_dt*128] -> [dt, p, ct, q]
        w = np.ascontiguousarray(
            w.reshape(n_ct, 128, n_dt, 128).transpose(2, 1, 0, 3))
        # fp8: scale weights by 64 so products sit in e4m3's sweet spot;
        # the scale cancels exactly in the LayerNorm that follows.
        return fp8(w, 64.0) if FP8_PROJ else bf(w)

    wq_p = pack_t(np.asarray(inputs["wq"], np.float32), H, HID // 128)
    wk_p = pack_t(np.asarray(inputs["wk"], np.float32), KV, HID // 128)
    wkc_p = pack_t(np.asarray(inputs["wk_cap"], np.float32), KV, CAP // 128)
    wv_b = bf(np.ascontiguousarray(
        np.asarray(inputs["wv"], np.float32)
        .reshape(HID // 128, 128, 4, KV * D // 4).transpose(2, 1, 0, 3)))
    wvc_b = bf(np.asarray(inputs["wv_cap"], np.float32))
    wo_b = bf(np.asarray(inputs["wo"], np.float32))

    lns = {}
    triv = []
    for nm, wk_, bk_ in (("q", "q_ln_w", "q_ln_b"), ("k", "k_ln_w", "k_ln_b"),
                         ("kc", "kc_ln_w", "kc_ln_b")):
        w = np.ascontiguousarray(np.asarray(inputs[wk_], np.float32))
        b = np.ascontiguousarray(np.asarray(inputs[bk_], np.float32))
        triv.append(bool(np.all(w == 1.0) and np.all(b == 0.0)))
        lns[f"ln_{nm}_w"] = w
        lns[f"ln_{nm}_b"] = b

    sign = np.tile([-1.0, 1.0], D // 2).astype(np.float32)
    # LN means are linear in x: row_mean(x @ W) = x @ row_sum(W)/N
    wq_s = np.asarray(inputs["wq"], np.float32).sum(axis=1) / (H * D)
    wk_s = np.asarray(inputs["wk"], np.float32).sum(axis=1) / (KV * D)
    wkc_s = np.asarray(inputs["wk_cap"], np.float32).sum(axis=1) / (KV * D)
    in_maps = []
    for c in range(NCORES):
        b_, half = divmod(c, 2)
        xTb = bf(x[b_].T)
        rows = slice(half * SQ, (half + 1) * SQ)
        cosT = np.repeat(cos[rows], 2, axis=1).T      # [128, SQ]
        sinT = (np.repeat(sin[rows], 2, axis=1) * sign).T
        mq_v = (x[b_, rows] @ wq_s)[None, :].repeat(128, axis=0)
        mk_v = (x[b_, rows] @ wk_s)[None, :].repeat(128, axis=0)
        mkc_v = (cap[b_] @ wkc_s)[None, :].repeat(128, axis=0)
        m = dict(
            xT=np.ascontiguousarray(xTb[:, half * SQ:(half + 1) * SQ]),
            capT=bf(cap[b_].T),
            cosT=bf(cosT),
            sinT=bf(sinT),
            wq=wq_p, wk=wk_p, wkc=wkc_p, wv=wv_b, wvc=wvc_b, wo=wo_b,
            m_q=bf(mq_v), m_k=bf(mk_v), m_kc=bf(mkc_v),
            **lns,
        )
        if FP8_PROJ:
            m["xT8"] = fp8(x[b_].T[:, half * SQ:(half + 1) * SQ])
            m["capT8"] = fp8(cap[b_].T)
        in_maps.append(m)
    return in_maps, gate_t, tuple(triv)


def _install_ntff_hook():
    """Shim the missing antenv.axon_hooks module so trace=True can capture
    NTFF profiles via the axon .so (test-time only)."""
    import types

    try:
        import antenv.axon_hooks  # noqa: F401
        return
    except ImportError:
        pass
    mod = types.ModuleType("antenv.axon_hooks")
    mod._hook = None

    def set_axon_ntff_profile_hook(h):
        mod._hook = h

    def get_axon_ntff_profile_hook():
        return mod._hook

    mod.set_axon_ntff_profile_hook = set_axon_ntff_profile_hook
    mod.get_axon_ntff_profile_hook = get_axon_ntff_profile_hook
    sys.modules["antenv.axon_hooks"] = mod
    import antenv
    antenv.axon_hooks = mod
    try:
        from trn_agent_boot.trn_boot import _ntff_profile_via_ctypes
        hook = _ntff_profile_via_ctypes("/opt/axon/libaxon_pjrt.so")
        if hook is not None:
            mod._hook = hook
    except Exception as e:  # degrade to no tracing
        print("ntff hook install failed:", e, file=sys.stderr)


def run_shards(cfg, inputs, trace=False):
    """Compile (cached), run on 8 cores, return (list of per-core outs, results)."""
    from concourse import bass_utils
    if trace:
        _install_ntff_hook()
    in_maps, gate_t, triv = make_in_maps(cfg, inputs)
    nc = _get_program(cfg, gate_t, triv)
    res = bass_utils.run_bass_kernel_spmd(
        nc, in_maps, core_ids=list(range(NCORES)), trace=trace)
    return [np.asarray(r["out"]).astype(np.float32) for r in res.results], res


def kernel(**inputs):
    outs, _ = run_shards(FULL_CFG, inputs, trace=False)
    SQ = FULL_CFG["SQ"]
    full = np.empty((B, S, HID), np.float32)
    for c in range(NCORES):
        b_, half = divmod(c, 2)
        full[b_, half * SQ:(half + 1) * SQ, :] = outs[c]
    return full



# revision 13
# speedup vs baseline: 1.0065x; 1.0065x over previous
"""Trainium2 Bass kernel for nn_Attention_58360015618558 (final).

Strategy (8 NeuronCores, SPMD). Measured 828 us (baseline 966 us),
rel err 7.8e-3.
  - Shard: core c -> (batch b = c//2, seq-half h = c%2); K/V computed for the
    local half and pair-AllGathered (2 MB each, hidden under compute).
  - Q/K/Kc projections are emitted DIRECTLY TRANSPOSED (weights stationary,
    x^T moving) so no PE transposes or PSUM->SBUF copies are needed.
    LayerNorm runs in the transposed layout: means come precomputed from the
    host (linear in x), E[z^2] via 1/N-scaled ones-matmuls (partition
    broadcast-reduce); RoPE's pair swap is a DVE stream_shuffle with rstd
    folded into transposed cos/sin tiles.  Projection order
    Vc, K, V, Q, Kc so the collectives and Q's long epilogue hide under
    later PE work.
  - Attention in transposed-score layout (S^T tiles): exp batched over
    [128,1024] 2-bank PSUM tiles (halves the ACT-overhead); softmax
    denominator via DVE pair-adds to 3 partials + ones-matmuls on PE; the
    caption denominator reuses the same PSUM bank.
  - Output projection interleaved per q-chunk into the attention loop (fills
    the PE while the scalar engine runs exp); output stored bf16, host upcast.
  - fp8 DoubleRow projections were measured at 5.2e-2 rel err (near-uniform
    attention passes q/k element noise straight through) — disabled.
"""

import math
import sys

import numpy as np

sys.path.insert(0, "/opt/trn_rl_repo")

import ml_dtypes  # noqa: E402

BF16 = ml_dtypes.bfloat16

# Full-size problem config
HID, H, KV, D, CAP = 2048, 16, 8, 128, 2048
B, S, LC = 4, 2048, 256
EPS = 1e-5
NCORES = 8

FULL_CFG = dict(S=S, SQ=S // 2, HID=HID, CAP=CAP, LC=LC, H=H, KV=KV)


DEBUG_DUMPS = False
# fp8e4 DoubleRow projections measured 5.2e-2 rel err (near-uniform attention
# probabilities pass q/k element noise straight to the output) — keep off.
FP8_PROJ = False


def _build(cfg, gate_t, ln_trivial=(True, True, True)):
    """Build the per-core Bass program. Returns compiled Bacc."""
    import concourse.bass as bass  # noqa: F401
    import concourse.mybir as mybir
    import concourse.tile as tile
    from concourse import bacc
    from contextlib import ExitStack

    FP = mybir.dt.float32
    BF = mybir.dt.bfloat16
    F8 = mybir.dt.float8e4
    WDT = F8 if FP8_PROJ else BF
    DR = mybir.MatmulPerfMode.DoubleRow
    AF = mybir.ActivationFunctionType
    ALU = mybir.AluOpType

    S_, SQ, HID_, CAP_, LC_ = cfg["S"], cfg["SQ"], cfg["HID"], cfg["CAP"], cfg["LC"]
    H_, KV_ = cfg["H"], cfg["KV"]
    HD, KD = H_ * D, KV_ * D
    CT, CTC = HID_ // 128, CAP_ // 128
    NQ, NK, NLC = SQ // 128, S_ // 128, LC_ // 128
    SCALE = 1.0 / math.sqrt(D)
    qtriv, ktriv, kctriv = ln_trivial
    SWAPMASK = [i ^ 1 for i in range(32)]

    nc = bacc.Bacc("TRN2", target_bir_lowering=False, debug=False,
                   num_devices=NCORES)

    def din(name, shape, dt=BF):
        return nc.dram_tensor(name, shape, dt, kind="ExternalInput").ap()

    # p-major packed inputs: single large DMAs instead of per-ct issues
    xT = din("xT", [128, CT, SQ])       # x[b].T columns, packed [p, ct, s]
    capT = din("capT", [128, CTC, LC_])
    # packed transposed-proj weights: [dt, p, ct, q]
    wq = din("wq", [HD // 128, 128, CT, 128], WDT)
    wk = din("wk", [KD // 128, 128, CT, 128], WDT)
    wkc = din("wkc", [KD // 128, 128, CTC, 128], WDT)
    if FP8_PROJ:
        xT8 = din("xT8", [HID_, SQ], F8)
        capT8 = din("capT8", [CAP_, LC_], F8)
    # natural-proj weights; wv packed as quarters [i, p, ct, 256]
    wv = din("wv", [4, 128, CT, KD // 4])
    wvc = din("wvc", [128, CTC, KD])
    wo = din("wo", [128, H_, HID_])
    cosT = din("cosT", [128, SQ])       # cosT[d,s] = cos[s, d//2]
    sinT = din("sinT", [128, SQ])       # signed: -sin even d, +sin odd d
    # LN means, precomputed host-side (linear in x) and pre-broadcast
    m_q = din("m_q", [128, SQ])
    m_k = din("m_k", [128, SQ])
    m_kc = din("m_kc", [128, LC_])
    lnw = {}
    for nm, dflat in (("q", HD), ("k", KD), ("kc", KD)):
        lnw[nm] = (din(f"ln_{nm}_w", [dflat], FP), din(f"ln_{nm}_b", [dflat], FP))
    out = nc.dram_tensor("out", [SQ, HID_], BF, kind="ExternalOutput").ap()

    with ExitStack() as top:
        tc = top.enter_context(tile.TileContext(nc))

        constp = top.enter_context(tc.tile_pool(name="const", bufs=1))
        resp = top.enter_context(tc.tile_pool(name="res", bufs=1))
        dramp = top.enter_context(tc.tile_pool(name="dram", bufs=1, space="DRAM"))

        ones_q = constp.tile([128, 128], BF, tag="ones_q", name="ones_q")
        nc.vector.memset(ones_q[:], 1.0 / HD)
        ones_k = constp.tile([128, 128], BF, tag="ones_k", name="ones_k")
        nc.vector.memset(ones_k[:], 1.0 / KD)
        ones_1 = constp.tile([128, 128], BF, tag="ones_1", name="ones_1")
        nc.vector.memset(ones_1[:], 1.0)
        zero_c = constp.tile([128, 1], FP, tag="zero_c", name="zero_c")
        nc.vector.memset(zero_c[:], 0.0)
        nc.const_aps.aps[(FP, 0.0)] = zero_c[:]
        eps_c = constp.tile([128, 1], FP, tag="eps_c", name="eps_c")
        nc.vector.memset(eps_c[:], EPS)
        nc.const_aps.aps[(FP, EPS)] = eps_c[:]

        # const loads on the gpsimd queue: keeps the Sync queue free for the
        # cap/wvc/xT streams that gate the first projections
        cos_res = constp.tile([128, SQ], BF, tag="cos_res", name="cos_res")
        sin_res = constp.tile([128, SQ], BF, tag="sin_res", name="sin_res")
        nc.gpsimd.dma_start(cos_res[:], cosT)
        nc.gpsimd.dma_start(sin_res[:], sinT)
        mq_res = constp.tile([128, SQ], BF, tag="mq_res", name="mq_res")
        mk_res = constp.tile([128, SQ], BF, tag="mk_res", name="mk_res")
        mkc_res = constp.tile([128, LC_], BF, tag="mkc_res", name="mkc_res")
        nc.gpsimd.dma_start(mq_res[:], m_q)
        nc.gpsimd.dma_start(mk_res[:], m_k)
        nc.gpsimd.dma_start(mkc_res[:], m_kc)

        # LN affine params in transposed layout: [128, n_dt] (col dt = head
        # tile), applied per-partition. Only loaded when nontrivial.
        affs = {}
        for nm, dflat, triv in (("q", HD, qtriv), ("k", KD, ktriv),
                                ("kc", KD, kctriv)):
            if not triv:
                wsb = constp.tile([128, dflat // 128], FP, tag=f"aw_{nm}",
                                  name=f"aw_{nm}")
                bsb = constp.tile([128, dflat // 128], FP, tag=f"ab_{nm}",
                                  name=f"ab_{nm}")
                nc.gpsimd.dma_start(wsb[:], lnw[nm][0].rearrange("(o p) -> p o", p=128))
                nc.gpsimd.dma_start(bsb[:], lnw[nm][1].rearrange("(o p) -> p o", p=128))
                affs[nm] = (wsb, bsb)

        # Resident tensors
        V_res = resp.tile([128, NK, KD], BF, tag="V_res", name="V_res")
        Vc_res = resp.tile([128, NLC, KD], BF, tag="Vc_res", name="Vc_res")
        KcT_res = resp.tile([128, KV_, LC_], BF, tag="KcT_res", name="KcT_res")
        QT_res = resp.tile([128, H_, SQ], BF, tag="QT_res", name="QT_res")

        # DRAM intermediates
        KVD = KV_ * 128
        KT_loc = dramp.tile([KVD, SQ], BF, tag="KT_loc", name="KT_loc")
        KT_g = dramp.tile([2 * KVD, SQ], BF, tag="KT_g", name="KT_g")
        V_loc = dramp.tile([SQ, KD], BF, tag="V_loc", name="V_loc")
        V_g = dramp.tile([2 * SQ, KD], BF, tag="V_g", name="V_g")
        aT = dramp.tile([H_, 128, SQ], BF, tag="aT", name="aT")
        aT_r = aT.rearrange("h p s -> p h s")
        dumps = {}
        if DEBUG_DUMPS:
            for nm, shp in (("d_qt", [H_, 128, SQ]), ("d_ktg", [2 * KVD, SQ]),
                            ("d_vg", [2 * SQ, KD]), ("d_at", [H_, 128, SQ]),
                            ("d_kct", [KV_, 128, LC_])):
                dumps[nm] = nc.dram_tensor(nm, shp, BF, kind="ExternalOutput").ap()
            for nm, shp, dt_ in (("d_rden", [128, 512], FP),
                                 ("d_rdenc", [128, 512], FP),
                                 ("d_t2", [128, 512], FP),
                                 ("d_tmp", [128, 512], FP),
                                 ("d_l2", [128, 4, 512], BF),
                                 ("d_pt0", [128, 1024], BF),
                                 ("d_ptc", [128, 1024], BF),
                                 ("d_denc", [128, 512], FP),
                                 ("d_poc", [128, 512], FP)):
                dumps[nm] = nc.dram_tensor(nm, shp, dt_,
                                           kind="ExternalOutput").ap()

        # ------------- transposed projection with LN (+RoPE) -------------
        def tproj(P, src_res, n_ct, w_ap, n_dt, s_len, ones_t, m_res,
                  rope, dest_fn, aff, dma_eng=None):
            """dest_fn(dt, s0, CW) -> ("sbuf", AP) | ("dram", AP).
            P: dict of shared pools."""
            CW = min(512, s_len)
            NCH = s_len // CW
            wtp, zp, zsqp, pp, statp, scp, app, stgp = (
                P["wt"], P["zp"], P["zsq"], P["pjps"], P["statp"], P["pjsc"],
                P["app"], P["stg"])

            for ch in range(NCH):
                s0 = ch * CW
                z = zp.tile([128, n_dt, CW], BF, tag="z", name="z")
                m_bf = m_res[:, s0:s0 + CW]
                msq = scp.tile([128, CW], FP, tag="msq", name="msq")
                nc.scalar.activation(msq[:], m_bf, AF.Square)
                sqP = statp.tile([128, CW], FP, tag="sqP", name="sqP")
                for dt in range(n_dt):
                    wt = wtp.tile([128, n_ct, 128], WDT, tag="wt", name="wt")
                    nc.sync.dma_start(wt[:], w_ap[dt])
                    ps = pp.tile([128, CW], FP, tag="ps", name="ps")
                    if FP8_PROJ:
                        for c2 in range(n_ct // 2):
                            nc.tensor.matmul(
                                ps[:], lhsT=wt[:, 2 * c2:2 * c2 + 2, :],
                                rhs=src_res[:, 2 * c2:2 * c2 + 2, s0:s0 + CW],
                                start=(c2 == 0), stop=(c2 == n_ct // 2 - 1),
                                perf_mode=DR)
                    else:
                        for ct in range(n_ct):
                            nc.tensor.matmul(ps[:], lhsT=wt[:, ct, :],
                                             rhs=src_res[:, ct, s0:s0 + CW],
                                             start=(ct == 0),
                                             stop=(ct == n_ct - 1))
                    nc.scalar.copy(z[:, dt, :], ps[:])
                    zq = zsqp.tile([128, CW], BF, tag="zq", name="zq")
                    nc.vector.tensor_tensor(zq[:], z[:, dt, :], z[:, dt, :],
                                            ALU.mult)
                    nc.tensor.matmul(sqP[:], lhsT=ones_t[:], rhs=zq[:],
                                     start=(dt == 0), stop=(dt == n_dt - 1))
                # stats: var = E[z^2] - m^2 (ones pre-scaled by 1/div)
                var = scp.tile([128, CW], FP, tag="var", name="var")
                nc.vector.tensor_tensor(var[:], sqP[:], msq[:], ALU.subtract)
                sd = scp.tile([128, CW], FP, tag="sd", name="sd")
                nc.scalar.activation(sd[:], var[:], AF.Sqrt, bias=EPS)
                rs = scp.tile([128, CW], FP, tag="rs", name="rs")
                nc.vector.reciprocal_approx_fast(rs[:], sd[:])
                if aff is None:
                    if rope:
                        RC = scp.tile([128, CW], BF, tag="RC", name="RC")
                        nc.vector.tensor_tensor(RC[:], rs[:],
                                                cos_res[:, s0:s0 + CW], ALU.mult)
                        RS = scp.tile([128, CW], BF, tag="RS", name="RS")
                        nc.vector.tensor_tensor(RS[:], rs[:],
                                                sin_res[:, s0:s0 + CW], ALU.mult)
                    else:
                        rs_bf = scp.tile([128, CW], BF, tag="rs_bf", name="rs_bf")
                        nc.vector.tensor_copy(rs_bf[:], rs[:])
                # zc pass first: frees z (and its SBUF region) early, before
                # the longer rope chains run on DVE
                zcs = []
                for dt in range(n_dt):
                    zc = app.tile([128, CW], BF, tag="zc", name="zc", bufs=16)
                    nc.vector.tensor_tensor(zc[:], z[:, dt, :], m_bf,
                                            ALU.subtract)
                    zcs.append(zc)
                for dt in range(n_dt):
                    zc = zcs[dt]
                    mode, dest = dest_fn(dt, s0, CW)
                    if mode == "dram":
                        o = stgp.tile([128, CW], BF, tag="o", name="o")
                        oap = o[:]
                    else:
                        oap = dest
                    if aff is not None:
                        # full chain: zn=zc*rs; za=zn*w+b; then rope
                        zn = app.tile([128, CW], BF, tag="zn", name="zn")
                        nc.vector.tensor_tensor(zn[:], zc[:], rs[:], ALU.mult)
                        za = app.tile([128, CW], BF, tag="za", name="za")
                        wsb, bsb = aff
                        nc.vector.tensor_scalar(za[:], zn[:],
                                                wsb[:, dt:dt + 1],
                                                bsb[:, dt:dt + 1],
                                                ALU.mult, ALU.add)
                        if rope:
                            sh = app.tile([128, CW], BF, tag="sh", name="sh")
                            nc.vector.stream_shuffle(sh[:], za[:], SWAPMASK)
                            t1 = app.tile([128, CW], BF, tag="t1", name="t1")
                            nc.vector.tensor_tensor(t1[:], za[:],
                                                    cos_res[:, s0:s0 + CW],
                                                    ALU.mult)
                            t2 = app.tile([128, CW], BF, tag="t2", name="t2")
                            nc.vector.tensor_tensor(t2[:], sh[:],
                                                    sin_res[:, s0:s0 + CW],
                                                    ALU.mult)
                            nc.vector.tensor_tensor(oap, t1[:], t2[:], ALU.add)
                        else:
                            nc.vector.tensor_copy(oap, za[:])
                    elif rope:
                        sh = app.tile([128, CW], BF, tag="sh", name="sh")
                        nc.vector.stream_shuffle(sh[:], zc[:], SWAPMASK)
                        t1 = app.tile([128, CW], BF, tag="t1", name="t1")
                        nc.vector.tensor_tensor(t1[:], zc[:], RC[:], ALU.mult)
                        t2 = app.tile([128, CW], BF, tag="t2", name="t2")
                        nc.vector.tensor_tensor(t2[:], sh[:], RS[:], ALU.mult)
                        nc.vector.tensor_tensor(oap, t1[:], t2[:], ALU.add)
                    else:
                        nc.vector.tensor_tensor(oap, zc[:], rs_bf[:], ALU.mult)
                    if mode == "dram":
                        (dma_eng or nc.sync).dma_start(dest, o[:])

        # ------------- natural projection (V / Vc), no LN -------------
        def nproj(P, src_res, n_ct, w_res, n_st, dest_fn):
            pp, stg = P["vps"], P["vstg"]
            for st in range(n_st):
                for i in range(KD // 512):
                    ps = pp.tile([128, 512], FP, tag="ps", name="ps")
                    for ct in range(n_ct):
                        nc.tensor.matmul(
                            ps[:], lhsT=src_res[:, ct, st * 128:(st + 1) * 128],
                            rhs=w_res[:, ct, i * 512:(i + 1) * 512],
                            start=(ct == 0), stop=(ct == n_ct - 1))
                    mode, dest = dest_fn(st, i)
                    if mode == "dram":
                        v = stg.tile([128, 512], BF, tag="v", name="v")
                        nc.scalar.copy(v[:], ps[:])
                        nc.sync.dma_start(dest, v[:])
                    else:
                        nc.scalar.copy(dest, ps[:])

        # ktw pool lives at top scope so phase-B K loads never wait on
        # phase-A SBUF frees; first two kv tiles are preloaded during A.
        kp = top.enter_context(tc.tile_pool(name="kw", bufs=2))

        def load_ktw(kv, eng=None):
            # eng=gpsimd for phase-A preloads: they depend on the K gather
            # and must not head-of-line-block the Sync queue
            eng = eng or nc.sync
            ktw = kp.tile([128, S_], BF, tag="ktw", name="ktw")
            eng.dma_start(ktw[:, 0:SQ], KT_g[kv * 128:(kv + 1) * 128, :])
            eng.dma_start(ktw[:, SQ:2 * SQ],
                          KT_g[KVD + kv * 128:KVD + (kv + 1) * 128, :])
            return ktw

        # ---------------- Phase A ----------------
        RG = [[2 * i, 2 * i + 1] for i in range(NCORES // 2)]
        ktw_pre = {}
        with ExitStack() as pa:
            # shared projection pools (one open/close for all of phase A)
            P = {}
            P["wt"] = pa.enter_context(tc.tile_pool(name="wt", bufs=2))
            P["zp"] = pa.enter_context(tc.tile_pool(name="zp", bufs=1))
            # (zp bufs=1 is safe: the zc-first pass frees z early each chunk)
            P["zsq"] = pa.enter_context(tc.tile_pool(name="zsq", bufs=2))
            P["pjps"] = pa.enter_context(tc.tile_pool(name="pjps", bufs=4,
                                                      space="PSUM"))
            P["statp"] = pa.enter_context(tc.tile_pool(name="statp", bufs=1,
                                                       space="PSUM"))
            P["pjsc"] = pa.enter_context(tc.tile_pool(name="pjsc", bufs=1))
            P["app"] = pa.enter_context(tc.tile_pool(name="app", bufs=2))
            P["stg"] = pa.enter_context(tc.tile_pool(name="stg", bufs=2))
            P["vps"] = pa.enter_context(tc.tile_pool(name="vps", bufs=3,
                                                     space="PSUM"))
            P["vstg"] = pa.enter_context(tc.tile_pool(name="vstg", bufs=3))

            # caption features stay resident: the (small) Kc projection runs
            # LAST so its short epilogue — not Q's long one — gates attention
            ctp = pa.enter_context(tc.tile_pool(name="ct", bufs=1))
            cap_res = ctp.tile([128, CTC, LC_], BF, tag="cap", name="cap")
            kc_src = cap_res

            with ExitStack() as s1:   # caption V weights
                wvcp = s1.enter_context(tc.tile_pool(name="wvcp", bufs=1))
                wvc_res = wvcp.tile([128, CTC, KD], BF, tag="wvc", name="wvc")
                nc.sync.dma_start(cap_res[:], capT)
                nc.sync.dma_start(wvc_res[:, 0:CTC // 2, :],
                                  wvc[:, 0:CTC // 2, :])
                nc.sync.dma_start(wvc_res[:, CTC // 2:CTC, :],
                                  wvc[:, CTC // 2:CTC, :])
                nproj(P, cap_res, CTC, wvc_res, NLC,
                      lambda st, i: ("sbuf",
                                     Vc_res[:, st, i * 512:(i + 1) * 512]))

            with ExitStack() as s2:   # x^T, through Q proj
                xtp = s2.enter_context(tc.tile_pool(name="xtq", bufs=1))
                x_src = xtp.tile([128, CT, SQ], BF, tag="xt", name="xt")
                nc.sync.dma_start(x_src[:, 0:CT // 2, :], xT[:, 0:CT // 2, :])
                nc.sync.dma_start(x_src[:, CT // 2:CT, :],
                                  xT[:, CT // 2:CT, :])
                tproj(P, x_src, CT, wk, KV_, SQ, ones_k, mk_res, rope=True,
                      dest_fn=lambda dt, s0, CW: (
                          "dram", KT_loc[dt * 128:(dt + 1) * 128, s0:s0 + CW]),
                      aff=affs.get("k"))
                nc.gpsimd.collective_compute(
                    "AllGather", ALU.bypass, replica_groups=RG,
                    ins=[KT_loc.opt()], outs=[KT_g.opt()])
                with ExitStack() as s3:   # V projection
                    wvp = s3.enter_context(tc.tile_pool(name="wvp", bufs=1))
                    # x^T resident (bf16), wv streamed in quarters
                    for i in range(4):
                        wvh = wvp.tile([128, CT, 256], BF, tag="wvh",
                                       name="wvh", bufs=2)
                        nc.sync.dma_start(wvh[:], wv[i])
                        for st in range(NQ):
                            ps = P["vps"].tile([128, 256], FP, tag="ps",
                                               name="ps")
                            for ct in range(CT):
                                nc.tensor.matmul(
                                    ps[:],
                                    lhsT=x_src[:, ct,
                                               st * 128:(st + 1) * 128],
                                    rhs=wvh[:, ct, :],
                                    start=(ct == 0), stop=(ct == CT - 1))
                            v = P["vstg"].tile([128, 256], BF, tag="v",
                                               name="v")
                            nc.scalar.copy(v[:], ps[:])
                            nc.gpsimd.dma_start(
                                V_loc[st * 128:(st + 1) * 128,
                                      i * 256:(i + 1) * 256], v[:])
                    nc.gpsimd.collective_compute(
                        "AllGather", ALU.bypass, replica_groups=RG,
                        ins=[V_loc.opt()], outs=[V_g.opt()])
                    # single strided fill off the Sync queue
                    nc.gpsimd.dma_start(
                        V_res[:],
                        V_g.rearrange("(nk p) kd -> p nk kd", p=128))
                # ktw preloads AFTER the V-path gpsimd traffic: they wait on
                # the K gather, and the gpsimd DMA queue is FIFO — putting
                # them earlier head-of-line-blocks the V_loc stores (measured
                # 40us PE stall + HAM re-throttle)
                for kvp in (0, 1):
                    ktw_pre[(0, kvp)] = load_ktw(kvp, eng=nc.gpsimd)
                tproj(P, x_src, CT, wq, H_, SQ, ones_q, mq_res, rope=True,
                      dest_fn=lambda dt, s0, CW: ("sbuf",
                                                  QT_res[:, dt, s0:s0 + CW]),
                      aff=affs.get("q"))
            # Kc last: its short epilogue gates attention start, while Q's
            # long rope epilogue drains during Kc's PE work
            tproj(P, kc_src, CTC, wkc, KV_, LC_, ones_k, mkc_res, rope=False,
                  dest_fn=lambda dt, s0, CW: ("sbuf",
                                              KcT_res[:, dt, s0:s0 + CW]),
                  aff=affs.get("kc"))
            # dummy exp: pulls the Sqrt->Exp ACT table-set switch (~2.7us)
            # into scalar-idle time instead of attention's first exp
            warm = constp.tile([128, 1], FP, tag="warm", name="warm")
            nc.scalar.activation(warm[:], zero_c[:], AF.Exp)

        if DEBUG_DUMPS:
            for hh in range(H_):
                nc.sync.dma_start(dumps["d_qt"][hh], QT_res[:, hh, :])
            for kk in range(KV_):
                nc.sync.dma_start(dumps["d_kct"][kk], KcT_res[:, kk, :])
            for kt_ in range(NK):
                nc.sync.dma_start(dumps["d_vg"][kt_ * 128:(kt_ + 1) * 128, :],
                                  V_res[:, kt_, :])

        # ---------------- Phase B + interleaved C ----------------
        QCH = 512
        NQC = SQ // QCH
        wop = top.enter_context(tc.tile_pool(name="wop", bufs=1))
        wo_res = wop.tile([128, H_, HID_], BF, tag="wo", name="wo")
        # gpsimd DMA queue: keeps these off the Sync queue's critical
        # path into attention (only needed by phase C)
        nc.gpsimd.dma_start(wo_res[:, 0:H_ // 2, :], wo[:, 0:H_ // 2, :])
        nc.gpsimd.dma_start(wo_res[:, H_ // 2:H_, :], wo[:, H_ // 2:H_, :])
        with ExitStack() as pb:
            ptp = pb.enter_context(tc.tile_pool(name="pt", bufs=3))
            l1p = pb.enter_context(tc.tile_pool(name="l1", bufs=4))
            l2p = pb.enter_context(tc.tile_pool(name="l2", bufs=8))
            epi = pb.enter_context(tc.tile_pool(name="epi", bufs=2))
            aop = pb.enter_context(tc.tile_pool(name="ao", bufs=2))
            ps_s = pb.enter_context(tc.tile_pool(name="ps_s", bufs=2, space="PSUM"))
            ps_o = pb.enter_context(tc.tile_pool(name="ps_o", bufs=1, space="PSUM"))
            ps_oc = pb.enter_context(tc.tile_pool(name="ps_oc", bufs=1,
                                                  space="PSUM"))
            ps_d = pb.enter_context(tc.tile_pool(name="ps_d", bufs=1, space="PSUM"))
            # phase C pools
            ap_ = pb.enter_context(tc.tile_pool(name="ast", bufs=2))
            op_ = pb.enter_context(tc.tile_pool(name="osb", bufs=2))
            cps = pb.enter_context(tc.tile_pool(name="cps", bufs=1, space="PSUM"))

            last_ao = [None]
            a_st_cur = [None]

            def emit_wo_unit(st, ec):
                # one (q-subtile, out-col-tile) of the output projection,
                # interleaved into the next chunk's attention so the PE
                # absorbs it while ACT is the bottleneck on exps
                if ec == 0:
                    a_st = ap_.tile([128, H_, 128], BF, tag="ast", name="ast")
                    nc.sync.dma_start(
                        a_st[:], aT_r[:, :, st * 128:(st + 1) * 128])
                    a_st_cur[0] = a_st
                a_st = a_st_cur[0]
                ps = cps.tile([128, 512], FP, tag="cps", name="cps")[:]
                for h in range(H_):
                    nc.tensor.matmul(ps, lhsT=a_st[:, h, :],
                                     rhs=wo_res[:, h,
                                                ec * 512:(ec + 1) * 512],
                                     start=(h == 0), stop=(h == H_ - 1))
                osb = op_.tile([128, 512], BF, tag="osb", name="osb")
                nc.vector.tensor_copy(osb[:], ps)
                nc.sync.dma_start(
                    out[st * 128:(st + 1) * 128, ec * 512:(ec + 1) * 512],
                    osb[:])

            units = []   # pending phase-C units of the previous chunk
            for ch in range(NQC):
                q0 = ch * QCH
                for kv in range(KV_):
                    ktw = ktw_pre.pop((ch, kv), None)
                    if ktw is None:
                        ktw = load_ktw(kv)
                    if DEBUG_DUMPS and ch == 0:
                        nc.sync.dma_start(
                            dumps["d_ktg"][kv * 128:(kv + 1) * 128, :],
                            ktw[:, 0:SQ])
                        nc.sync.dma_start(
                            dumps["d_ktg"][KVD + kv * 128:KVD + (kv + 1) * 128, :],
                            ktw[:, SQ:2 * SQ])
                    for rep in range(H_ // KV_):
                        h = kv * (H_ // KV_) + rep
                        qs = QT_res[:, h, q0:q0 + QCH]
                        po = ps_o.tile([128, QCH], FP, tag="po", name="po")
                        poc = ps_oc.tile([128, QCH], FP, tag="poc", name="poc")
                        pden = ps_d.tile([128, QCH], FP, tag="pden", name="pden")
                        l2s = []
                        for g in range(NK // 2):
                            ps2 = ps_s.tile([128, 2 * QCH], FP, tag="s", name="s")
                            nc.tensor.matmul(ps2[:, 0:QCH],
                                             lhsT=ktw[:, (2 * g) * 128:
                                                      (2 * g + 1) * 128],
                                             rhs=qs, start=True, stop=True)
                            nc.tensor.matmul(ps2[:, QCH:2 * QCH],
                                             lhsT=ktw[:, (2 * g + 1) * 128:
                                                      (2 * g + 2) * 128],
                                             rhs=qs, start=True, stop=True)
                            pt2 = ptp.tile([128, 2 * QCH], BF, tag="pt", name="pt")
                            nc.scalar.activation(pt2[:], ps2[:], AF.Exp,
                                                 scale=SCALE)
                            nc.tensor.matmul(
                                po[:], lhsT=V_res[:, 2 * g,
                                                  kv * 128:(kv + 1) * 128],
                                rhs=pt2[:, 0:QCH], start=(g == 0), stop=False)
                            nc.tensor.matmul(
                                po[:], lhsT=V_res[:, 2 * g + 1,
                                                  kv * 128:(kv + 1) * 128],
                                rhs=pt2[:, QCH:2 * QCH], start=False,
                                stop=(g == NK // 2 - 1))
                            if DEBUG_DUMPS and h == 0 and ch == 0 and g == 0:
                                nc.sync.dma_start(dumps["d_pt0"], pt2[:])
                            l1 = l1p.tile([128, QCH], BF, tag="l1", name="l1")
                            nc.vector.tensor_tensor(l1[:], pt2[:, 0:QCH],
                                                    pt2[:, QCH:2 * QCH], ALU.add)
                            l2s.append(l1)
                            # combine pairs except the last one, so the pden
                            # matmuls needn't wait for the final DVE add
                            if g % 2 == 1 and g != NK // 2 - 1:
                                l2 = l2p.tile([128, QCH], BF, tag="l2", name="l2")
                                nc.vector.tensor_tensor(l2[:], l2s[-2][:],
                                                        l2s[-1][:], ALU.add)
                                l2s = l2s[:-2] + [None]
                                l2s[-1] = l2
                            # two extra mid-iteration combines (inputs already
                            # available) cut the pden matmuls from 5 to 3
                            if g in (3, 6) and len(l2s) >= 2:
                                l3 = l2p.tile([128, QCH], BF, tag="l2", name="l2")
                                nc.vector.tensor_tensor(l3[:], l2s[-2][:],
                                                        l2s[-1][:], ALU.add)
                                l2s = l2s[:-2] + [l3]
                        # pden = sum of the 4 level-2 partials: ones-matmuls
                        # broadcast-reduce over partitions (keeps the DVE
                        # epilogue chain short)
                        l2fin = [t for t in l2s if t is not None]
                        # caption scores first: they are independent of the
                        # DVE partial-sum tree, so the scalar engine gets its
                        # next exp sooner while pden waits on the tree
                        psc = ps_s.tile([128, 2 * QCH], FP, tag="s", name="s")
                        nc.tensor.matmul(psc[:, 0:QCH],
                                         lhsT=KcT_res[:, kv, 0:128],
                                         rhs=qs, start=True, stop=True)
                        nc.tensor.matmul(psc[:, QCH:2 * QCH],
                                         lhsT=KcT_res[:, kv, 128:256],
                                         rhs=qs, start=True, stop=True)
                        ptc = ptp.tile([128, 2 * QCH], BF, tag="pt", name="pt")
                        nc.scalar.activation(ptc[:], psc[:], AF.Exp, scale=SCALE)
                        for j, l2 in enumerate(l2fin):
                            nc.tensor.matmul(pden[:], lhsT=ones_1[:], rhs=l2[:],
                                             start=(j == 0),
                                             stop=(j == len(l2fin) - 1))
                        nc.tensor.matmul(poc[:],
                                         lhsT=Vc_res[:, 0, kv * 128:(kv + 1) * 128],
                                         rhs=ptc[:, 0:QCH], start=True, stop=False)
                        nc.tensor.matmul(poc[:],
                                         lhsT=Vc_res[:, 1, kv * 128:(kv + 1) * 128],
                                         rhs=ptc[:, QCH:2 * QCH], start=False,
                                         stop=True)
                        # caption pair-sum (elementwise); partition reduction
                        # happens below via a ones-matmul reusing pden's bank
                        denc = epi.tile([128, QCH], BF, tag="denc", name="denc")
                        nc.vector.tensor_tensor(denc[:], ptc[:, 0:QCH],
                                                ptc[:, QCH:2 * QCH], ALU.add)
                        if DEBUG_DUMPS and h == 0 and ch == 0:
                            nc.sync.dma_start(dumps["d_ptc"], ptc[:])
                            pocc = epi.tile([128, QCH], FP, tag="pocc",
                                            name="pocc")
                            nc.vector.tensor_copy(pocc[:], poc[:])
                            nc.sync.dma_start(dumps["d_poc"], pocc[:])
                        # epilogue
                        rden = epi.tile([128, QCH], FP, tag="rden", name="rden")
                        nc.vector.reciprocal_approx_fast(rden[:], pden[:])
                        nc.tensor.matmul(pden[:], lhsT=ones_1[:], rhs=denc[:],
                                         start=True, stop=True)
                        rdenc = epi.tile([128, QCH], FP, tag="rdenc", name="rdenc")
                        nc.vector.reciprocal_approx_fast(rdenc[:], pden[:])
                        if DEBUG_DUMPS and h == 0 and ch == 0:
                            nc.sync.dma_start(dumps["d_denc"], rdenc[:])
                        t2 = epi.tile([128, QCH], FP, tag="t2", name="t2")
                        nc.vector.tensor_tensor(t2[:], po[:], rden[:], ALU.mult)
                        tmp = epi.tile([128, QCH], FP, tag="tmp", name="tmp")
                        nc.vector.scalar_tensor_tensor(
                            tmp[:], poc[:], float(gate_t[h]), rdenc[:],
                            ALU.mult, ALU.mult)
                        if DEBUG_DUMPS and h == 0 and ch == 0:
                            nc.sync.dma_start(dumps["d_rden"], rden[:])
                            nc.sync.dma_start(dumps["d_rdenc"], rdenc[:])
                            nc.sync.dma_start(dumps["d_t2"], t2[:])
                            nc.sync.dma_start(dumps["d_tmp"], tmp[:])
                        ao = aop.tile([128, QCH], BF, tag="ao", name="ao")
                        nc.vector.tensor_tensor(ao[:], t2[:], tmp[:], ALU.add)
                        nc.sync.dma_start(aT[h, :, q0:q0 + QCH], ao[:])
                        if h == H_ - 1:
                            last_ao[0] = ao
                        if DEBUG_DUMPS:
                            nc.sync.dma_start(dumps["d_at"][h, :, q0:q0 + QCH],
                                              ao[:])
                        if units:
                            emit_wo_unit(*units.pop(0))
                if ch < NQC - 1:
                    # queue this chunk's phase C for interleave into the next
                    # chunk's attention (16 units, 16 head-iters: exact fit)
                    units = [(st, ec)
                             for st in range(q0 // 128, (q0 + QCH) // 128)
                             for ec in range(HID_ // 512)]
                    continue
                # ---- tail phase C (last chunk only) ----
                # Nothing left to overlap: double-buffer the accumulator by
                # borrowing score-pool banks.
                for st2, ec2 in units:   # leftovers (shouldn't happen)
                    emit_wo_unit(st2, ec2)
                tail = True
                for st in range(q0 // 128, (q0 + QCH) // 128):
                    a_st = ap_.tile([128, H_, 128], BF, tag="ast", name="ast")
                    if tail:
                        # last head's output read straight from SBUF: skips
                        # waiting on its DRAM round trip
                        nc.sync.dma_start(
                            a_st[:, 0:H_ - 1, :],
                            aT_r[:, 0:H_ - 1, st * 128:(st + 1) * 128])
                        o0 = st * 128 - q0
                        nc.vector.tensor_copy(a_st[:, H_ - 1, :],
                                              last_ao[0][:, o0:o0 + 128])
                    else:
                        nc.sync.dma_start(
                            a_st[:], aT_r[:, :, st * 128:(st + 1) * 128])
                    for ec in range(HID_ // 512):
                        if tail and ec % 2 == 1:
                            psfull = ps_s.tile([128, 2 * QCH], FP, tag="s",
                                               name="s")
                            ps = psfull[:, 0:512]
                        else:
                            ps = cps.tile([128, 512], FP, tag="cps",
                                          name="cps")[:]
                        for h in range(H_):
                            nc.tensor.matmul(ps, lhsT=a_st[:, h, :],
                                             rhs=wo_res[:, h,
                                                        ec * 512:(ec + 1) * 512],
                                             start=(h == 0), stop=(h == H_ - 1))
                        osb = op_.tile([128, 512], BF, tag="osb", name="osb")
                        nc.vector.tensor_copy(osb[:], ps)
                        nc.sync.dma_start(
                            out[st * 128:(st + 1) * 128, ec * 512:(ec + 1) * 512],
                            osb[:])

    nc.compile()
    return nc


_CACHE = {}


def _get_program(cfg, gate_t, ln_trivial):
    key = (tuple(sorted(cfg.items())), tuple(np.round(gate_t, 8)), ln_trivial,
           FP8_PROJ)
    if key not in _CACHE:
        _CACHE[key] = _build(cfg, gate_t, ln_trivial)
    return _CACHE[key]


def make_in_maps(cfg, inputs):
    """Host-side sharding: returns (in_maps, gate_t, ln_trivial)."""
    S_, SQ = cfg["S"], cfg["SQ"]
    x = np.asarray(inputs["x"], np.float32)
    cap = np.asarray(inputs["caption_feat"], np.float32)
    cos = np.ascontiguousarray(np.asarray(inputs["freqs_cos"], np.float32))
    sin = np.ascontiguousarray(np.asarray(inputs["freqs_sin"], np.float32))
    gate_t = np.tanh(np.asarray(inputs["gate"], np.float32))

    F8 = ml_dtypes.float8_e4m3

    def bf(a):
        return np.ascontiguousarray(a).astype(BF16)

    def fp8(a, scale=1.0):
        return np.ascontiguousarray(
            np.clip(a * scale, -440.0, 440.0)).astype(F8)

    def pack_t(w, n_dt, n_ct):
        # [n_ct*128, n_dt*128] -> [dt, p, ct, q]
        w = np.ascontiguousarray(
            w.reshape(n_ct, 128, n_dt, 128).transpose(2, 1, 0, 3))
        # fp8: scale weights by 64 so products sit in e4m3's sweet spot;
        # the scale cancels exactly in the LayerNorm that follows.
        return fp8(w, 64.0) if FP8_PROJ else bf(w)

    def pmajor(w, n_ct):
        # [n_ct*128, d] -> [128, n_ct, d] (single contiguous DMA per tile)
        return bf(w.reshape(n_ct, 128, -1).transpose(1, 0, 2))

    wq_p = pack_t(np.asarray(inputs["wq"], np.float32), H, HID // 128)
    wk_p = pack_t(np.asarray(inputs["wk"], np.float32), KV, HID // 128)
    wkc_p = pack_t(np.asarray(inputs["wk_cap"], np.float32), KV, CAP // 128)
    wv_b = bf(np.ascontiguousarray(
        np.asarray(inputs["wv"], np.float32)
        .reshape(HID // 128, 128, 4, KV * D // 4).transpose(2, 1, 0, 3)))
    wvc_b = pmajor(np.asarray(inputs["wv_cap"], np.float32), CAP // 128)
    wo_b = pmajor(np.asarray(inputs["wo"], np.float32), H)

    lns = {}
    triv = []
    for nm, wk_, bk_ in (("q", "q_ln_w", "q_ln_b"), ("k", "k_ln_w", "k_ln_b"),
                         ("kc", "kc_ln_w", "kc_ln_b")):
        w = np.ascontiguousarray(np.asarray(inputs[wk_], np.float32))
        b = np.ascontiguousarray(np.asarray(inputs[bk_], np.float32))
        triv.append(bool(np.all(w == 1.0) and np.all(b == 0.0)))
        lns[f"ln_{nm}_w"] = w
        lns[f"ln_{nm}_b"] = b

    sign = np.tile([-1.0, 1.0], D // 2).astype(np.float32)
    # LN means are linear in x: row_mean(x @ W) = x @ row_sum(W)/N
    wq_s = np.asarray(inputs["wq"], np.float32).sum(axis=1) / (H * D)
    wk_s = np.asarray(inputs["wk"], np.float32).sum(axis=1) / (KV * D)
    wkc_s = np.asarray(inputs["wk_cap"], np.float32).sum(axis=1) / (KV * D)
    in_maps = []
    for c in range(NCORES):
        b_, half = divmod(c, 2)
        rows = slice(half * SQ, (half + 1) * SQ)
        cosT = np.repeat(cos[rows], 2, axis=1).T      # [128, SQ]
        sinT = (np.repeat(sin[rows], 2, axis=1) * sign).T
        mq_v = (x[b_, rows] @ wq_s)[None, :].repeat(128, axis=0)
        mk_v = (x[b_, rows] @ wk_s)[None, :].repeat(128, axis=0)
        mkc_v = (cap[b_] @ wkc_s)[None, :].repeat(128, axis=0)
        m = dict(
            xT=pmajor(np.ascontiguousarray(x[b_].T[:, rows]), HID // 128),
            capT=pmajor(np.ascontiguousarray(cap[b_].T), CAP // 128),
            cosT=bf(cosT),
            sinT=bf(sinT),
            wq=wq_p, wk=wk_p, wkc=wkc_p, wv=wv_b, wvc=wvc_b, wo=wo_b,
            m_q=bf(mq_v), m_k=bf(mk_v), m_kc=bf(mkc_v),
            **lns,
        )
        in_maps.append(m)
    return in_maps, gate_t, tuple(triv)


def _install_ntff_hook():
    """Shim the missing antenv.axon_hooks module so trace=True can capture
    NTFF profiles via the axon .so (test-time only)."""
    import types

    try:
        import antenv.axon_hooks  # noqa: F401
        return
    except ImportError:
        pass
    mod = types.ModuleType("antenv.axon_hooks")
    mod._hook = None

    def set_axon_ntff_profile_hook(h):
        mod._hook = h

    def get_axon_ntff_profile_hook():
        return mod._hook

    mod.set_axon_ntff_profile_hook = set_axon_ntff_profile_hook
    mod.get_axon_ntff_profile_hook = get_axon_ntff_profile_hook
    sys.modules["antenv.axon_hooks"] = mod
    import antenv
    antenv.axon_hooks = mod
    try:
        from trn_agent_boot.trn_boot import _ntff_profile_via_ctypes
        hook = _ntff_profile_via_ctypes("/opt/axon/libaxon_pjrt.so")
        if hook is not None:
            mod._hook = hook
    except Exception as e:  # degrade to no tracing
        print("ntff hook install failed:", e, file=sys.stderr)


def run_shards(cfg, inputs, trace=False):
    """Compile (cached), run on 8 cores, return (list of per-core outs, results)."""
    from concourse import bass_utils
    if trace:
        _install_ntff_hook()
    in_maps, gate_t, triv = make_in_maps(cfg, inputs)
    nc = _get_program(cfg, gate_t, triv)
    res = bass_utils.run_bass_kernel_spmd(
        nc, in_maps, core_ids=list(range(NCORES)), trace=trace)
    return [np.asarray(r["out"]).astype(np.float32) for r in res.results], res


def kernel(**inputs):
    outs, _ = run_shards(FULL_CFG, inputs, trace=False)
    SQ = FULL_CFG["SQ"]
    full = np.empty((B, S, HID), np.float32)
    for c in range(NCORES):
        b_, half = divmod(c, 2)
        full[b_, half * SQ:(half + 1) * SQ, :] = outs[c]
    return full



# revision 21
# speedup vs baseline: 1.0474x; 1.0406x over previous
"""Trainium2 Bass kernel for nn_Attention_58360015618558 (final).

Strategy (8 NeuronCores, SPMD). Measured 828 us (baseline 966 us),
rel err 7.8e-3.
  - Shard: core c -> (batch b = c//2, seq-half h = c%2); K/V computed for the
    local half and pair-AllGathered (2 MB each, hidden under compute).
  - Q/K/Kc projections are emitted DIRECTLY TRANSPOSED (weights stationary,
    x^T moving) so no PE transposes or PSUM->SBUF copies are needed.
    LayerNorm runs in the transposed layout: means come precomputed from the
    host (linear in x), E[z^2] via 1/N-scaled ones-matmuls (partition
    broadcast-reduce); RoPE's pair swap is a DVE stream_shuffle with rstd
    folded into transposed cos/sin tiles.  Projection order
    Vc, K, V, Q, Kc so the collectives and Q's long epilogue hide under
    later PE work.
  - Attention in transposed-score layout (S^T tiles): exp batched over
    [128,1024] 2-bank PSUM tiles (halves the ACT-overhead); softmax
    denominator via DVE pair-adds to 3 partials + ones-matmuls on PE; the
    caption denominator reuses the same PSUM bank.
  - Output projection interleaved per q-chunk into the attention loop (fills
    the PE while the scalar engine runs exp); output stored bf16, host upcast.
  - fp8 DoubleRow projections were measured at 5.2e-2 rel err (near-uniform
    attention passes q/k element noise straight through) — disabled.
"""

import math
import sys

import numpy as np

sys.path.insert(0, "/opt/trn_rl_repo")

import ml_dtypes  # noqa: E402

BF16 = ml_dtypes.bfloat16

# Full-size problem config
HID, H, KV, D, CAP = 2048, 16, 8, 128, 2048
B, S, LC = 4, 2048, 256
EPS = 1e-5
NCORES = 8

FULL_CFG = dict(S=S, SQ=S // 2, HID=HID, CAP=CAP, LC=LC, H=H, KV=KV)


DEBUG_DUMPS = False
# fp8e4 DoubleRow projections measured 5.2e-2 rel err (near-uniform attention
# probabilities pass q/k element noise straight to the output) — keep off.
FP8_PROJ = False


def _build(cfg, gate_t, ln_trivial=(True, True, True)):
    """Build the per-core Bass program. Returns compiled Bacc."""
    import concourse.bass as bass  # noqa: F401
    import concourse.mybir as mybir
    import concourse.tile as tile
    from concourse import bacc
    from contextlib import ExitStack

    FP = mybir.dt.float32
    BF = mybir.dt.bfloat16
    F8 = mybir.dt.float8e4
    WDT = F8 if FP8_PROJ else BF
    DR = mybir.MatmulPerfMode.DoubleRow
    AF = mybir.ActivationFunctionType
    ALU = mybir.AluOpType

    S_, SQ, HID_, CAP_, LC_ = cfg["S"], cfg["SQ"], cfg["HID"], cfg["CAP"], cfg["LC"]
    H_, KV_ = cfg["H"], cfg["KV"]
    HD, KD = H_ * D, KV_ * D
    CT, CTC = HID_ // 128, CAP_ // 128
    NQ, NK, NLC = SQ // 128, S_ // 128, LC_ // 128
    SCALE = 1.0 / math.sqrt(D)
    qtriv, ktriv, kctriv = ln_trivial
    SWAPMASK = [i ^ 1 for i in range(32)]

    nc = bacc.Bacc("TRN2", target_bir_lowering=False, debug=False,
                   num_devices=NCORES)

    def din(name, shape, dt=BF):
        return nc.dram_tensor(name, shape, dt, kind="ExternalInput").ap()

    # p-major packed inputs: single large DMAs instead of per-ct issues
    xT = din("xT", [128, CT, SQ])       # x[b].T columns, packed [p, ct, s]
    capT = din("capT", [128, CTC, LC_])
    # packed transposed-proj weights: [dt, p, ct, q]
    wq = din("wq", [HD // 128, 128, CT, 128], WDT)
    wk = din("wk", [KD // 128, 128, CT, 128], WDT)
    wkc = din("wkc", [KD // 128, 128, CTC, 128], WDT)
    if FP8_PROJ:
        xT8 = din("xT8", [HID_, SQ], F8)
        capT8 = din("capT8", [CAP_, LC_], F8)
    # natural-proj weights; wv packed as quarters [i, p, ct, 256]
    wv = din("wv", [4, 128, CT, KD // 4])
    wvc = din("wvc", [128, CTC, KD])
    wo = din("wo", [128, H_, HID_])
    cosT = din("cosT", [128, SQ])       # cosT[d,s] = cos[s, d//2]
    sinT = din("sinT", [128, SQ])       # signed: -sin even d, +sin odd d
    # LN means, precomputed host-side (linear in x) and pre-broadcast
    m_q = din("m_q", [128, SQ])
    m_k = din("m_k", [128, SQ])
    m_kc = din("m_kc", [128, LC_])
    lnw = {}
    for nm, dflat in (("q", HD), ("k", KD), ("kc", KD)):
        lnw[nm] = (din(f"ln_{nm}_w", [dflat], FP), din(f"ln_{nm}_b", [dflat], FP))
    out = nc.dram_tensor("out", [SQ, HID_], BF, kind="ExternalOutput").ap()

    with ExitStack() as top:
        tc = top.enter_context(tile.TileContext(nc))

        constp = top.enter_context(tc.tile_pool(name="const", bufs=1))
        resp = top.enter_context(tc.tile_pool(name="res", bufs=1))
        dramp = top.enter_context(tc.tile_pool(name="dram", bufs=1, space="DRAM"))

        ones_q = constp.tile([128, 128], BF, tag="ones_q", name="ones_q")
        nc.vector.memset(ones_q[:], 1.0 / HD)
        ones_k = constp.tile([128, 128], BF, tag="ones_k", name="ones_k")
        nc.vector.memset(ones_k[:], 1.0 / KD)
        ones_1 = constp.tile([128, 128], BF, tag="ones_1", name="ones_1")
        nc.vector.memset(ones_1[:], 1.0)
        zero_c = constp.tile([128, 1], FP, tag="zero_c", name="zero_c")
        nc.vector.memset(zero_c[:], 0.0)
        nc.const_aps.aps[(FP, 0.0)] = zero_c[:]
        eps_c = constp.tile([128, 1], FP, tag="eps_c", name="eps_c")
        nc.vector.memset(eps_c[:], EPS)
        nc.const_aps.aps[(FP, EPS)] = eps_c[:]

        # const loads on the gpsimd queue: keeps the Sync queue free for the
        # cap/wvc/xT streams that gate the first projections
        cos_res = constp.tile([128, SQ], BF, tag="cos_res", name="cos_res")
        sin_res = constp.tile([128, SQ], BF, tag="sin_res", name="sin_res")
        nc.gpsimd.dma_start(cos_res[:], cosT)
        nc.gpsimd.dma_start(sin_res[:], sinT)
        mq_res = constp.tile([128, SQ], BF, tag="mq_res", name="mq_res")
        mk_res = constp.tile([128, SQ], BF, tag="mk_res", name="mk_res")
        mkc_res = constp.tile([128, LC_], BF, tag="mkc_res", name="mkc_res")
        nc.gpsimd.dma_start(mq_res[:], m_q)
        nc.gpsimd.dma_start(mk_res[:], m_k)
        nc.gpsimd.dma_start(mkc_res[:], m_kc)

        # LN affine params in transposed layout: [128, n_dt] (col dt = head
        # tile), applied per-partition. Only loaded when nontrivial.
        affs = {}
        for nm, dflat, triv in (("q", HD, qtriv), ("k", KD, ktriv),
                                ("kc", KD, kctriv)):
            if not triv:
                wsb = constp.tile([128, dflat // 128], FP, tag=f"aw_{nm}",
                                  name=f"aw_{nm}")
                bsb = constp.tile([128, dflat // 128], FP, tag=f"ab_{nm}",
                                  name=f"ab_{nm}")
                nc.gpsimd.dma_start(wsb[:], lnw[nm][0].rearrange("(o p) -> p o", p=128))
                nc.gpsimd.dma_start(bsb[:], lnw[nm][1].rearrange("(o p) -> p o", p=128))
                affs[nm] = (wsb, bsb)

        # Resident tensors
        V_res = resp.tile([128, NK, KD], BF, tag="V_res", name="V_res")
        Vc_res = resp.tile([128, NLC, KD], BF, tag="Vc_res", name="Vc_res")
        KcT_res = resp.tile([128, KV_, LC_], BF, tag="KcT_res", name="KcT_res")
        QT_res = resp.tile([128, H_, SQ], BF, tag="QT_res", name="QT_res")

        # DRAM intermediates
        KVD = KV_ * 128
        KT_loc = dramp.tile([KVD, SQ], BF, tag="KT_loc", name="KT_loc")
        KT_g = dramp.tile([2 * KVD, SQ], BF, tag="KT_g", name="KT_g")
        V_loc = dramp.tile([SQ, KD], BF, tag="V_loc", name="V_loc")
        V_g = dramp.tile([2 * SQ, KD], BF, tag="V_g", name="V_g")
        aT = dramp.tile([H_, 128, SQ], BF, tag="aT", name="aT")
        aT_r = aT.rearrange("h p s -> p h s")
        dumps = {}
        if DEBUG_DUMPS:
            for nm, shp in (("d_qt", [H_, 128, SQ]), ("d_ktg", [2 * KVD, SQ]),
                            ("d_vg", [2 * SQ, KD]), ("d_at", [H_, 128, SQ]),
                            ("d_kct", [KV_, 128, LC_])):
                dumps[nm] = nc.dram_tensor(nm, shp, BF, kind="ExternalOutput").ap()
            for nm, shp, dt_ in (("d_rden", [128, 512], FP),
                                 ("d_rdenc", [128, 512], FP),
                                 ("d_t2", [128, 512], FP),
                                 ("d_tmp", [128, 512], FP),
                                 ("d_l2", [128, 4, 512], BF),
                                 ("d_pt0", [128, 1024], BF),
                                 ("d_ptc", [128, 1024], BF),
                                 ("d_denc", [128, 512], FP),
                                 ("d_poc", [128, 512], FP)):
                dumps[nm] = nc.dram_tensor(nm, shp, dt_,
                                           kind="ExternalOutput").ap()

        # ------------- transposed projection with LN (+RoPE) -------------
        def tproj(P, src_res, n_ct, w_ap, n_dt, s_len, ones_t, m_res,
                  rope, dest_fn, aff, dma_eng=None):
            """dest_fn(dt, s0, CW) -> ("sbuf", AP) | ("dram", AP).
            P: dict of shared pools."""
            CW = min(512, s_len)
            NCH = s_len // CW
            wtp, zp, zsqp, pp, statp, scp, app, stgp = (
                P["wt"], P["zp"], P["zsq"], P["pjps"], P["statp"], P["pjsc"],
                P["app"], P["stg"])

            for ch in range(NCH):
                s0 = ch * CW
                z = zp.tile([128, n_dt, CW], BF, tag="z", name="z")
                m_bf = m_res[:, s0:s0 + CW]
                msq = scp.tile([128, CW], FP, tag="msq", name="msq")
                nc.scalar.activation(msq[:], m_bf, AF.Square)
                sqP = statp.tile([128, CW], FP, tag="sqP", name="sqP")
                for dt in range(n_dt):
                    wt = wtp.tile([128, n_ct, 128], WDT, tag="wt", name="wt")
                    nc.sync.dma_start(wt[:], w_ap[dt])
                    ps = pp.tile([128, CW], FP, tag="ps", name="ps")
                    if FP8_PROJ:
                        for c2 in range(n_ct // 2):
                            nc.tensor.matmul(
                                ps[:], lhsT=wt[:, 2 * c2:2 * c2 + 2, :],
                                rhs=src_res[:, 2 * c2:2 * c2 + 2, s0:s0 + CW],
                                start=(c2 == 0), stop=(c2 == n_ct // 2 - 1),
                                perf_mode=DR)
                    else:
                        for ct in range(n_ct):
                            nc.tensor.matmul(ps[:], lhsT=wt[:, ct, :],
                                             rhs=src_res[:, ct, s0:s0 + CW],
                                             start=(ct == 0),
                                             stop=(ct == n_ct - 1))
                    nc.scalar.copy(z[:, dt, :], ps[:])
                    zq = zsqp.tile([128, CW], BF, tag="zq", name="zq")
                    nc.vector.tensor_tensor(zq[:], z[:, dt, :], z[:, dt, :],
                                            ALU.mult)
                    nc.tensor.matmul(sqP[:], lhsT=ones_t[:], rhs=zq[:],
                                     start=(dt == 0), stop=(dt == n_dt - 1))
                # stats: var = E[z^2] - m^2 (ones pre-scaled by 1/div)
                var = scp.tile([128, CW], FP, tag="var", name="var")
                nc.vector.tensor_tensor(var[:], sqP[:], msq[:], ALU.subtract)
                sd = scp.tile([128, CW], FP, tag="sd", name="sd")
                nc.scalar.activation(sd[:], var[:], AF.Sqrt, bias=EPS)
                rs = scp.tile([128, CW], FP, tag="rs", name="rs")
                nc.vector.reciprocal_approx_fast(rs[:], sd[:])
                if aff is None:
                    if rope:
                        RC = scp.tile([128, CW], BF, tag="RC", name="RC")
                        nc.vector.tensor_tensor(RC[:], rs[:],
                                                cos_res[:, s0:s0 + CW], ALU.mult)
                        RS = scp.tile([128, CW], BF, tag="RS", name="RS")
                        nc.vector.tensor_tensor(RS[:], rs[:],
                                                sin_res[:, s0:s0 + CW], ALU.mult)
                    else:
                        rs_bf = scp.tile([128, CW], BF, tag="rs_bf", name="rs_bf")
                        nc.vector.tensor_copy(rs_bf[:], rs[:])
                # zc pass first: frees z (and its SBUF region) early, before
                # the longer rope chains run on DVE
                zcs = []
                for dt in range(n_dt):
                    zc = app.tile([128, CW], BF, tag="zc", name="zc", bufs=12)
                    nc.vector.tensor_tensor(zc[:], z[:, dt, :], m_bf,
                                            ALU.subtract)
                    zcs.append(zc)
                for dt in range(n_dt):
                    zc = zcs[dt]
                    mode, dest = dest_fn(dt, s0, CW)
                    if mode == "dram":
                        o = stgp.tile([128, CW], BF, tag="o", name="o")
                        oap = o[:]
                    else:
                        oap = dest
                    if aff is not None:
                        # full chain: zn=zc*rs; za=zn*w+b; then rope
                        zn = app.tile([128, CW], BF, tag="zn", name="zn")
                        nc.vector.tensor_tensor(zn[:], zc[:], rs[:], ALU.mult)
                        za = app.tile([128, CW], BF, tag="za", name="za")
                        wsb, bsb = aff
                        nc.vector.tensor_scalar(za[:], zn[:],
                                                wsb[:, dt:dt + 1],
                                                bsb[:, dt:dt + 1],
                                                ALU.mult, ALU.add)
                        if rope:
                            sh = app.tile([128, CW], BF, tag="sh", name="sh")
                            nc.vector.stream_shuffle(sh[:], za[:], SWAPMASK)
                            t1 = app.tile([128, CW], BF, tag="t1", name="t1")
                            nc.vector.tensor_tensor(t1[:], za[:],
                                                    cos_res[:, s0:s0 + CW],
                                                    ALU.mult)
                            t2 = app.tile([128, CW], BF, tag="t2", name="t2")
                            nc.vector.tensor_tensor(t2[:], sh[:],
                                                    sin_res[:, s0:s0 + CW],
                                                    ALU.mult)
                            nc.vector.tensor_tensor(oap, t1[:], t2[:], ALU.add)
                        else:
                            nc.vector.tensor_copy(oap, za[:])
                    elif rope:
                        sh = app.tile([128, CW], BF, tag="sh", name="sh")
                        nc.vector.stream_shuffle(sh[:], zc[:], SWAPMASK)
                        t1 = app.tile([128, CW], BF, tag="t1", name="t1")
                        nc.vector.tensor_tensor(t1[:], zc[:], RC[:], ALU.mult)
                        t2 = app.tile([128, CW], BF, tag="t2", name="t2")
                        nc.vector.tensor_tensor(t2[:], sh[:], RS[:], ALU.mult)
                        nc.vector.tensor_tensor(oap, t1[:], t2[:], ALU.add)
                    else:
                        nc.vector.tensor_tensor(oap, zc[:], rs_bf[:], ALU.mult)
                    if mode == "dram":
                        (dma_eng or nc.sync).dma_start(dest, o[:])

        # ------------- natural projection (V / Vc), no LN -------------
        def nproj(P, src_res, n_ct, w_res, n_st, dest_fn):
            pp, stg = P["vps"], P["vstg"]
            for st in range(n_st):
                for i in range(KD // 512):
                    ps = pp.tile([128, 512], FP, tag="ps", name="ps")
                    for ct in range(n_ct):
                        nc.tensor.matmul(
                            ps[:], lhsT=src_res[:, ct, st * 128:(st + 1) * 128],
                            rhs=w_res[:, ct, i * 512:(i + 1) * 512],
                            start=(ct == 0), stop=(ct == n_ct - 1))
                    mode, dest = dest_fn(st, i)
                    if mode == "dram":
                        v = stg.tile([128, 512], BF, tag="v", name="v")
                        nc.scalar.copy(v[:], ps[:])
                        nc.sync.dma_start(dest, v[:])
                    else:
                        nc.scalar.copy(dest, ps[:])

        # ktw pool lives at top scope so phase-B K loads never wait on
        # phase-A SBUF frees; first two kv tiles are preloaded during A.
        kp = top.enter_context(tc.tile_pool(name="kw", bufs=2))

        def load_ktw(kv, eng=None):
            # eng=gpsimd for phase-A preloads: they depend on the K gather
            # and must not head-of-line-block the Sync queue
            eng = eng or nc.sync
            ktw = kp.tile([128, S_], BF, tag="ktw", name="ktw")
            eng.dma_start(ktw[:, 0:SQ], KT_g[kv * 128:(kv + 1) * 128, :])
            eng.dma_start(ktw[:, SQ:2 * SQ],
                          KT_g[KVD + kv * 128:KVD + (kv + 1) * 128, :])
            return ktw

        # ---------------- Phase A ----------------
        RG = [[2 * i, 2 * i + 1] for i in range(NCORES // 2)]
        ktw_pre = {}
        with ExitStack() as pa:
            # shared projection pools (one open/close for all of phase A)
            P = {}
            # bufs=3: weight loads land well ahead of their matmuls even when
            # the DMA engines are congested by a concurrent collective
            P["wt"] = pa.enter_context(tc.tile_pool(name="wt", bufs=3))
            P["zp"] = pa.enter_context(tc.tile_pool(name="zp", bufs=1))
            # (zp bufs=1 is safe: the zc-first pass frees z early each chunk)
            P["zsq"] = pa.enter_context(tc.tile_pool(name="zsq", bufs=2))
            P["pjps"] = pa.enter_context(tc.tile_pool(name="pjps", bufs=4,
                                                      space="PSUM"))
            P["statp"] = pa.enter_context(tc.tile_pool(name="statp", bufs=1,
                                                       space="PSUM"))
            P["pjsc"] = pa.enter_context(tc.tile_pool(name="pjsc", bufs=1))
            P["app"] = pa.enter_context(tc.tile_pool(name="app", bufs=2))
            P["stg"] = pa.enter_context(tc.tile_pool(name="stg", bufs=2))
            P["vps"] = pa.enter_context(tc.tile_pool(name="vps", bufs=3,
                                                     space="PSUM"))
            P["vstg"] = pa.enter_context(tc.tile_pool(name="vstg", bufs=3))

            # caption features stay resident: the (small) Kc projection runs
            # LAST so its short epilogue — not Q's long one — gates attention
            ctp = pa.enter_context(tc.tile_pool(name="ct", bufs=1))
            cap_res = ctp.tile([128, CTC, LC_], BF, tag="cap", name="cap")
            kc_src = cap_res

            with ExitStack() as s1:   # caption V weights
                wvcp = s1.enter_context(tc.tile_pool(name="wvcp", bufs=1))
                wvc_res = wvcp.tile([128, CTC, KD], BF, tag="wvc", name="wvc")
                nc.sync.dma_start(cap_res[:], capT)
                nc.sync.dma_start(wvc_res[:, 0:CTC // 2, :],
                                  wvc[:, 0:CTC // 2, :])
                nc.sync.dma_start(wvc_res[:, CTC // 2:CTC, :],
                                  wvc[:, CTC // 2:CTC, :])
                nproj(P, cap_res, CTC, wvc_res, NLC,
                      lambda st, i: ("sbuf",
                                     Vc_res[:, st, i * 512:(i + 1) * 512]))

            with ExitStack() as s2:   # x^T, through Q proj
                xtp = s2.enter_context(tc.tile_pool(name="xtq", bufs=1))
                x_src = xtp.tile([128, CT, SQ], BF, tag="xt", name="xt")
                nc.sync.dma_start(x_src[:, 0:CT // 2, :], xT[:, 0:CT // 2, :])
                nc.sync.dma_start(x_src[:, CT // 2:CT, :],
                                  xT[:, CT // 2:CT, :])
                tproj(P, x_src, CT, wk, KV_, SQ, ones_k, mk_res, rope=True,
                      dest_fn=lambda dt, s0, CW: (
                          "dram", KT_loc[dt * 128:(dt + 1) * 128, s0:s0 + CW]),
                      aff=affs.get("k"))
                nc.gpsimd.collective_compute(
                    "AllGather", ALU.bypass, replica_groups=RG,
                    ins=[KT_loc.opt()], outs=[KT_g.opt()])
                with ExitStack() as s3:   # V projection
                    wvp = s3.enter_context(tc.tile_pool(name="wvp", bufs=1))
                    # x^T resident (bf16), wv streamed in quarters
                    for i in range(4):
                        wvh = wvp.tile([128, CT, 256], BF, tag="wvh",
                                       name="wvh", bufs=2)
                        nc.sync.dma_start(wvh[:], wv[i])
                        for st in range(NQ):
                            ps = P["vps"].tile([128, 256], FP, tag="ps",
                                               name="ps")
                            for ct in range(CT):
                                nc.tensor.matmul(
                                    ps[:],
                                    lhsT=x_src[:, ct,
                                               st * 128:(st + 1) * 128],
                                    rhs=wvh[:, ct, :],
                                    start=(ct == 0), stop=(ct == CT - 1))
                            v = P["vstg"].tile([128, 256], BF, tag="v",
                                               name="v")
                            nc.scalar.copy(v[:], ps[:])
                            # sync queue: the gpsimd queue is blocked by the
                            # in-flight K AllGather (collective_compute stalls
                            # its queue until completion)
                            nc.sync.dma_start(
                                V_loc[st * 128:(st + 1) * 128,
                                      i * 256:(i + 1) * 256], v[:])
                    nc.gpsimd.collective_compute(
                        "AllGather", ALU.bypass, replica_groups=RG,
                        ins=[V_loc.opt()], outs=[V_g.opt()])
                    # single strided fill off the Sync queue
                    nc.gpsimd.dma_start(
                        V_res[:],
                        V_g.rearrange("(nk p) kd -> p nk kd", p=128))
                # ktw preloads AFTER the V-path gpsimd traffic: they wait on
                # the K gather, and the gpsimd DMA queue is FIFO — putting
                # them earlier head-of-line-blocks the V fill (measured
                # 40us PE stall + HAM re-throttle)
                for kvp in (0, 1):
                    ktw_pre[(0, kvp)] = load_ktw(kvp, eng=nc.gpsimd)
                tproj(P, x_src, CT, wq, H_, SQ, ones_q, mq_res, rope=True,
                      dest_fn=lambda dt, s0, CW: ("sbuf",
                                                  QT_res[:, dt, s0:s0 + CW]),
                      aff=affs.get("q"))
            # Kc last: its short epilogue gates attention start, while Q's
            # long rope epilogue drains during Kc's PE work
            tproj(P, kc_src, CTC, wkc, KV_, LC_, ones_k, mkc_res, rope=False,
                  dest_fn=lambda dt, s0, CW: ("sbuf",
                                              KcT_res[:, dt, s0:s0 + CW]),
                  aff=affs.get("kc"))
            # dummy exp: pulls the Sqrt->Exp ACT table-set switch (~2.7us)
            # into scalar-idle time instead of attention's first exp
            warm = constp.tile([128, 1], FP, tag="warm", name="warm")
            nc.scalar.activation(warm[:], zero_c[:], AF.Exp)

        if DEBUG_DUMPS:
            for hh in range(H_):
                nc.sync.dma_start(dumps["d_qt"][hh], QT_res[:, hh, :])
            for kk in range(KV_):
                nc.sync.dma_start(dumps["d_kct"][kk], KcT_res[:, kk, :])
            for kt_ in range(NK):
                nc.sync.dma_start(dumps["d_vg"][kt_ * 128:(kt_ + 1) * 128, :],
                                  V_res[:, kt_, :])

        # ---------------- Phase B + interleaved C ----------------
        QCH = 512
        NQC = SQ // QCH
        wop = top.enter_context(tc.tile_pool(name="wop", bufs=1))
        wo_res = wop.tile([128, H_, HID_], BF, tag="wo", name="wo")
        # gpsimd DMA queue: keeps these off the Sync queue's critical
        # path into attention (only needed by phase C)
        nc.gpsimd.dma_start(wo_res[:, 0:H_ // 2, :], wo[:, 0:H_ // 2, :])
        nc.gpsimd.dma_start(wo_res[:, H_ // 2:H_, :], wo[:, H_ // 2:H_, :])
        with ExitStack() as pb:
            ptp = pb.enter_context(tc.tile_pool(name="pt", bufs=3))
            l1p = pb.enter_context(tc.tile_pool(name="l1", bufs=4))
            l2p = pb.enter_context(tc.tile_pool(name="l2", bufs=8))
            epi = pb.enter_context(tc.tile_pool(name="epi", bufs=2))
            aop = pb.enter_context(tc.tile_pool(name="ao", bufs=2))
            ps_s = pb.enter_context(tc.tile_pool(name="ps_s", bufs=2, space="PSUM"))
            ps_o = pb.enter_context(tc.tile_pool(name="ps_o", bufs=1, space="PSUM"))
            ps_oc = pb.enter_context(tc.tile_pool(name="ps_oc", bufs=1,
                                                  space="PSUM"))
            ps_d = pb.enter_context(tc.tile_pool(name="ps_d", bufs=1, space="PSUM"))
            # phase C pools
            ap_ = pb.enter_context(tc.tile_pool(name="ast", bufs=2))
            op_ = pb.enter_context(tc.tile_pool(name="osb", bufs=2))
            cps = pb.enter_context(tc.tile_pool(name="cps", bufs=1, space="PSUM"))

            last_ao = [None]
            a_st_cur = [None]

            def emit_wo_unit(st, ec):
                # one (q-subtile, out-col-tile) of the output projection,
                # interleaved into the next chunk's attention so the PE
                # absorbs it while ACT is the bottleneck on exps
                if ec == 0:
                    a_st = ap_.tile([128, H_, 128], BF, tag="ast", name="ast")
                    nc.sync.dma_start(
                        a_st[:], aT_r[:, :, st * 128:(st + 1) * 128])
                    a_st_cur[0] = a_st
                a_st = a_st_cur[0]
                ps = cps.tile([128, 512], FP, tag="cps", name="cps")[:]
                for h in range(H_):
                    nc.tensor.matmul(ps, lhsT=a_st[:, h, :],
                                     rhs=wo_res[:, h,
                                                ec * 512:(ec + 1) * 512],
                                     start=(h == 0), stop=(h == H_ - 1))
                osb = op_.tile([128, 512], BF, tag="osb", name="osb")
                nc.vector.tensor_copy(osb[:], ps)
                nc.sync.dma_start(
                    out[st * 128:(st + 1) * 128, ec * 512:(ec + 1) * 512],
                    osb[:])

            units = []   # pending phase-C units of the previous chunk
            for ch in range(NQC):
                q0 = ch * QCH
                for kv in range(KV_):
                    ktw = ktw_pre.pop((ch, kv), None)
                    if ktw is None:
                        ktw = load_ktw(kv)
                    if DEBUG_DUMPS and ch == 0:
                        nc.sync.dma_start(
                            dumps["d_ktg"][kv * 128:(kv + 1) * 128, :],
                            ktw[:, 0:SQ])
                        nc.sync.dma_start(
                            dumps["d_ktg"][KVD + kv * 128:KVD + (kv + 1) * 128, :],
                            ktw[:, SQ:2 * SQ])
                    for rep in range(H_ // KV_):
                        h = kv * (H_ // KV_) + rep
                        qs = QT_res[:, h, q0:q0 + QCH]
                        po = ps_o.tile([128, QCH], FP, tag="po", name="po")
                        poc = ps_oc.tile([128, QCH], FP, tag="poc", name="poc")
                        pden = ps_d.tile([128, QCH], FP, tag="pden", name="pden")
                        l2s = []
                        for g in range(NK // 2):
                            ps2 = ps_s.tile([128, 2 * QCH], FP, tag="s", name="s")
                            nc.tensor.matmul(ps2[:, 0:QCH],
                                             lhsT=ktw[:, (2 * g) * 128:
                                                      (2 * g + 1) * 128],
                                             rhs=qs, start=True, stop=True)
                            nc.tensor.matmul(ps2[:, QCH:2 * QCH],
                                             lhsT=ktw[:, (2 * g + 1) * 128:
                                                      (2 * g + 2) * 128],
                                             rhs=qs, start=True, stop=True)
                            pt2 = ptp.tile([128, 2 * QCH], BF, tag="pt", name="pt")
                            nc.scalar.activation(pt2[:], ps2[:], AF.Exp,
                                                 scale=SCALE)
                            nc.tensor.matmul(
                                po[:], lhsT=V_res[:, 2 * g,
                                                  kv * 128:(kv + 1) * 128],
                                rhs=pt2[:, 0:QCH], start=(g == 0), stop=False)
                            nc.tensor.matmul(
                                po[:], lhsT=V_res[:, 2 * g + 1,
                                                  kv * 128:(kv + 1) * 128],
                                rhs=pt2[:, QCH:2 * QCH], start=False,
                                stop=(g == NK // 2 - 1))
                            if DEBUG_DUMPS and h == 0 and ch == 0 and g == 0:
                                nc.sync.dma_start(dumps["d_pt0"], pt2[:])
                            l1 = l1p.tile([128, QCH], BF, tag="l1", name="l1")
                            nc.vector.tensor_tensor(l1[:], pt2[:, 0:QCH],
                                                    pt2[:, QCH:2 * QCH], ALU.add)
                            l2s.append(l1)
                            # combine pairs except the last one, so the pden
                            # matmuls needn't wait for the final DVE add
                            if g % 2 == 1 and g != NK // 2 - 1:
                                l2 = l2p.tile([128, QCH], BF, tag="l2", name="l2")
                                nc.vector.tensor_tensor(l2[:], l2s[-2][:],
                                                        l2s[-1][:], ALU.add)
                                l2s = l2s[:-2] + [None]
                                l2s[-1] = l2
                            # two extra mid-iteration combines (inputs already
                            # available) cut the pden matmuls from 5 to 3
                            if g in (3, 6) and len(l2s) >= 2:
                                l3 = l2p.tile([128, QCH], BF, tag="l2", name="l2")
                                nc.vector.tensor_tensor(l3[:], l2s[-2][:],
                                                        l2s[-1][:], ALU.add)
                                l2s = l2s[:-2] + [l3]
                        # pden = sum of the 4 level-2 partials: ones-matmuls
                        # broadcast-reduce over partitions (keeps the DVE
                        # epilogue chain short)
                        l2fin = [t for t in l2s if t is not None]
                        # caption scores first: they are independent of the
                        # DVE partial-sum tree, so the scalar engine gets its
                        # next exp sooner while pden waits on the tree
                        psc = ps_s.tile([128, 2 * QCH], FP, tag="s", name="s")
                        nc.tensor.matmul(psc[:, 0:QCH],
                                         lhsT=KcT_res[:, kv, 0:128],
                                         rhs=qs, start=True, stop=True)
                        nc.tensor.matmul(psc[:, QCH:2 * QCH],
                                         lhsT=KcT_res[:, kv, 128:256],
                                         rhs=qs, start=True, stop=True)
                        ptc = ptp.tile([128, 2 * QCH], BF, tag="pt", name="pt")
                        nc.scalar.activation(ptc[:], psc[:], AF.Exp, scale=SCALE)
                        for j, l2 in enumerate(l2fin):
                            nc.tensor.matmul(pden[:], lhsT=ones_1[:], rhs=l2[:],
                                             start=(j == 0),
                                             stop=(j == len(l2fin) - 1))
                        nc.tensor.matmul(poc[:],
                                         lhsT=Vc_res[:, 0, kv * 128:(kv + 1) * 128],
                                         rhs=ptc[:, 0:QCH], start=True, stop=False)
                        nc.tensor.matmul(poc[:],
                                         lhsT=Vc_res[:, 1, kv * 128:(kv + 1) * 128],
                                         rhs=ptc[:, QCH:2 * QCH], start=False,
                                         stop=True)
                        # caption pair-sum (elementwise); partition reduction
                        # happens below via a ones-matmul reusing pden's bank
                        denc = epi.tile([128, QCH], BF, tag="denc", name="denc")
                        nc.vector.tensor_tensor(denc[:], ptc[:, 0:QCH],
                                                ptc[:, QCH:2 * QCH], ALU.add)
                        if DEBUG_DUMPS and h == 0 and ch == 0:
                            nc.sync.dma_start(dumps["d_ptc"], ptc[:])
                            pocc = epi.tile([128, QCH], FP, tag="pocc",
                                            name="pocc")
                            nc.vector.tensor_copy(pocc[:], poc[:])
                            nc.sync.dma_start(dumps["d_poc"], pocc[:])
                        # epilogue
                        rden = epi.tile([128, QCH], FP, tag="rden", name="rden")
                        nc.vector.reciprocal_approx_fast(rden[:], pden[:])
                        nc.tensor.matmul(pden[:], lhsT=ones_1[:], rhs=denc[:],
                                         start=True, stop=True)
                        rdenc = epi.tile([128, QCH], FP, tag="rdenc", name="rdenc")
                        nc.vector.reciprocal_approx_fast(rdenc[:], pden[:])
                        if DEBUG_DUMPS and h == 0 and ch == 0:
                            nc.sync.dma_start(dumps["d_denc"], rdenc[:])
                        t2 = epi.tile([128, QCH], FP, tag="t2", name="t2")
                        nc.vector.tensor_tensor(t2[:], po[:], rden[:], ALU.mult)
                        tmp = epi.tile([128, QCH], FP, tag="tmp", name="tmp")
                        nc.vector.scalar_tensor_tensor(
                            tmp[:], poc[:], float(gate_t[h]), rdenc[:],
                            ALU.mult, ALU.mult)
                        if DEBUG_DUMPS and h == 0 and ch == 0:
                            nc.sync.dma_start(dumps["d_rden"], rden[:])
                            nc.sync.dma_start(dumps["d_rdenc"], rdenc[:])
                            nc.sync.dma_start(dumps["d_t2"], t2[:])
                            nc.sync.dma_start(dumps["d_tmp"], tmp[:])
                        ao = aop.tile([128, QCH], BF, tag="ao", name="ao")
                        nc.vector.tensor_tensor(ao[:], t2[:], tmp[:], ALU.add)
                        nc.sync.dma_start(aT[h, :, q0:q0 + QCH], ao[:])
                        if h == H_ - 1:
                            last_ao[0] = ao
                        if DEBUG_DUMPS:
                            nc.sync.dma_start(dumps["d_at"][h, :, q0:q0 + QCH],
                                              ao[:])
                        if units:
                            emit_wo_unit(*units.pop(0))
                if ch < NQC - 1:
                    # queue this chunk's phase C for interleave into the next
                    # chunk's attention (16 units, 16 head-iters: exact fit)
                    units = [(st, ec)
                             for st in range(q0 // 128, (q0 + QCH) // 128)
                             for ec in range(HID_ // 512)]
                    continue
                # ---- tail phase C (last chunk only) ----
                # Nothing left to overlap: double-buffer the accumulator by
                # borrowing score-pool banks.
                for st2, ec2 in units:   # leftovers (shouldn't happen)
                    emit_wo_unit(st2, ec2)
                tail = True
                for st in range(q0 // 128, (q0 + QCH) // 128):
                    a_st = ap_.tile([128, H_, 128], BF, tag="ast", name="ast")
                    if tail:
                        # last head's output read straight from SBUF: skips
                        # waiting on its DRAM round trip
                        nc.sync.dma_start(
                            a_st[:, 0:H_ - 1, :],
                            aT_r[:, 0:H_ - 1, st * 128:(st + 1) * 128])
                        o0 = st * 128 - q0
                        nc.vector.tensor_copy(a_st[:, H_ - 1, :],
                                              last_ao[0][:, o0:o0 + 128])
                    else:
                        nc.sync.dma_start(
                            a_st[:], aT_r[:, :, st * 128:(st + 1) * 128])
                    for ec in range(HID_ // 512):
                        if tail and ec % 2 == 1:
                            psfull = ps_s.tile([128, 2 * QCH], FP, tag="s",
                                               name="s")
                            ps = psfull[:, 0:512]
                        else:
                            ps = cps.tile([128, 512], FP, tag="cps",
                                          name="cps")[:]
                        for h in range(H_):
                            nc.tensor.matmul(ps, lhsT=a_st[:, h, :],
                                             rhs=wo_res[:, h,
                                                        ec * 512:(ec + 1) * 512],
                                             start=(h == 0), stop=(h == H_ - 1))
                        osb = op_.tile([128, 512], BF, tag="osb", name="osb")
                        nc.vector.tensor_copy(osb[:], ps)
                        nc.sync.dma_start(
                            out[st * 128:(st + 1) * 128, ec * 512:(ec + 1) * 512],
                            osb[:])

    nc.compile()
    return nc


_CACHE = {}


def _get_program(cfg, gate_t, ln_trivial):
    key = (tuple(sorted(cfg.items())), tuple(np.round(gate_t, 8)), ln_trivial,
           FP8_PROJ)
    if key not in _CACHE:
        _CACHE[key] = _build(cfg, gate_t, ln_trivial)
    return _CACHE[key]


def make_in_maps(cfg, inputs):
    """Host-side sharding: returns (in_maps, gate_t, ln_trivial)."""
    S_, SQ = cfg["S"], cfg["SQ"]
    x = np.asarray(inputs["x"], np.float32)
    cap = np.asarray(inputs["caption_feat"], np.float32)
    cos = np.ascontiguousarray(np.asarray(inputs["freqs_cos"], np.float32))
    sin = np.ascontiguousarray(np.asarray(inputs["freqs_sin"], np.float32))
    gate_t = np.tanh(np.asarray(inputs["gate"], np.float32))

    F8 = ml_dtypes.float8_e4m3

    def bf(a):
        return np.ascontiguousarray(a).astype(BF16)

    def fp8(a, scale=1.0):
        return np.ascontiguousarray(
            np.clip(a * scale, -440.0, 440.0)).astype(F8)

    def pack_t(w, n_dt, n_ct):
        # [n_ct*128, n_dt*128] -> [dt, p, ct, q]
        w = np.ascontiguousarray(
            w.reshape(n_ct, 128, n_dt, 128).transpose(2, 1, 0, 3))
        # fp8: scale weights by 64 so products sit in e4m3's sweet spot;
        # the scale cancels exactly in the LayerNorm that follows.
        return fp8(w, 64.0) if FP8_PROJ else bf(w)

    def pmajor(w, n_ct):
        # [n_ct*128, d] -> [128, n_ct, d] (single contiguous DMA per tile)
        return bf(w.reshape(n_ct, 128, -1).transpose(1, 0, 2))

    wq_p = pack_t(np.asarray(inputs["wq"], np.float32), H, HID // 128)
    wk_p = pack_t(np.asarray(inputs["wk"], np.float32), KV, HID // 128)
    wkc_p = pack_t(np.asarray(inputs["wk_cap"], np.float32), KV, CAP // 128)
    wv_b = bf(np.ascontiguousarray(
        np.asarray(inputs["wv"], np.float32)
        .reshape(HID // 128, 128, 4, KV * D // 4).transpose(2, 1, 0, 3)))
    wvc_b = pmajor(np.asarray(inputs["wv_cap"], np.float32), CAP // 128)
    wo_b = pmajor(np.asarray(inputs["wo"], np.float32), H)

    lns = {}
    triv = []
    for nm, wk_, bk_ in (("q", "q_ln_w", "q_ln_b"), ("k", "k_ln_w", "k_ln_b"),
                         ("kc", "kc_ln_w", "kc_ln_b")):
        w = np.ascontiguousarray(np.asarray(inputs[wk_], np.float32))
        b = np.ascontiguousarray(np.asarray(inputs[bk_], np.float32))
        triv.append(bool(np.all(w == 1.0) and np.all(b == 0.0)))
        lns[f"ln_{nm}_w"] = w
        lns[f"ln_{nm}_b"] = b

    sign = np.tile([-1.0, 1.0], D // 2).astype(np.float32)
    # LN means are linear in x: row_mean(x @ W) = x @ row_sum(W)/N
    wq_s = np.asarray(inputs["wq"], np.float32).sum(axis=1) / (H * D)
    wk_s = np.asarray(inputs["wk"], np.float32).sum(axis=1) / (KV * D)
    wkc_s = np.asarray(inputs["wk_cap"], np.float32).sum(axis=1) / (KV * D)
    in_maps = []
    for c in range(NCORES):
        b_, half = divmod(c, 2)
        rows = slice(half * SQ, (half + 1) * SQ)
        cosT = np.repeat(cos[rows], 2, axis=1).T      # [128, SQ]
        sinT = (np.repeat(sin[rows], 2, axis=1) * sign).T
        mq_v = (x[b_, rows] @ wq_s)[None, :].repeat(128, axis=0)
        mk_v = (x[b_, rows] @ wk_s)[None, :].repeat(128, axis=0)
        mkc_v = (cap[b_] @ wkc_s)[None, :].repeat(128, axis=0)
        m = dict(
            xT=pmajor(np.ascontiguousarray(x[b_].T[:, rows]), HID // 128),
            capT=pmajor(np.ascontiguousarray(cap[b_].T), CAP // 128),
            cosT=bf(cosT),
            sinT=bf(sinT),
            wq=wq_p, wk=wk_p, wkc=wkc_p, wv=wv_b, wvc=wvc_b, wo=wo_b,
            m_q=bf(mq_v), m_k=bf(mk_v), m_kc=bf(mkc_v),
            **lns,
        )
        in_maps.append(m)
    return in_maps, gate_t, tuple(triv)


def _install_ntff_hook():
    """Shim the missing antenv.axon_hooks module so trace=True can capture
    NTFF profiles via the axon .so (test-time only)."""
    import types

    try:
        import antenv.axon_hooks  # noqa: F401
        return
    except ImportError:
        pass
    mod = types.ModuleType("antenv.axon_hooks")
    mod._hook = None

    def set_axon_ntff_profile_hook(h):
        mod._hook = h

    def get_axon_ntff_profile_hook():
        return mod._hook

    mod.set_axon_ntff_profile_hook = set_axon_ntff_profile_hook
    mod.get_axon_ntff_profile_hook = get_axon_ntff_profile_hook
    sys.modules["antenv.axon_hooks"] = mod
    import antenv
    antenv.axon_hooks = mod
    try:
        from trn_agent_boot.trn_boot import _ntff_profile_via_ctypes
        hook = _ntff_profile_via_ctypes("/opt/axon/libaxon_pjrt.so")
        if hook is not None:
            mod._hook = hook
    except Exception as e:  # degrade to no tracing
        print("ntff hook install failed:", e, file=sys.stderr)


def run_shards(cfg, inputs, trace=False):
    """Compile (cached), run on 8 cores, return (list of per-core outs, results)."""
    from concourse import bass_utils
    if trace:
        _install_ntff_hook()
    in_maps, gate_t, triv = make_in_maps(cfg, inputs)
    nc = _get_program(cfg, gate_t, triv)
    res = bass_utils.run_bass_kernel_spmd(
        nc, in_maps, core_ids=list(range(NCORES)), trace=trace)
    return [np.asarray(r["out"]).astype(np.float32) for r in res.results], res


def kernel(**inputs):
    outs, _ = run_shards(FULL_CFG, inputs, trace=False)
    SQ = FULL_CFG["SQ"]
    full = np.empty((B, S, HID), np.float32)
    for c in range(NCORES):
        b_, half = divmod(c, 2)
        full[b_, half * SQ:(half + 1) * SQ, :] = outs[c]
    return full

